# revision 1
# baseline (speedup 1.0000x reference)
"""Bahdanau-style attention kernel for Trainium2 (8 NeuronCores, SPMD).

Math (per batch row b):
    h_proj = hidden @ a_w[:DEC]                       (DEC,)
    e_proj[s, :] = enc[s, :] @ a_w[DEC:]              (S, DEC)
    energy = tanh(e_proj + h_proj + a_b)              (S, DEC)
    scores = energy @ v_w                             (S,)
    scores = where(mask == 0, -1e10, scores)
    attn = softmax(scores)                            (S,)
    out = attn @ enc                                  (ENC,)

Sharding: data-parallel over batch (32 rows -> 4 rows on each of 8 cores);
weights replicated.

Per-core strategy (dense path, SPARSE=False — see note above P_PAD for the
optional on-device mask-compaction path):
  - Encoder outputs are DMA-loaded with an fp32->bf16 cast (SWDGE) in four
    512-token chunks per batch row, kept in natural (s, e) layout for the
    final weighted sum.
  - e_proj is computed transposed (d on partitions, tokens on free dim; the
    (e, tok) operand comes from the DMA xbar transpose) so that
    (h_proj + a_b) is a per-partition scalar -> one ScalarE activation does
    bias + tanh while evacuating PSUM.
  - scores = v . tanh is a K=128 M=1 matmul; the attn row is transposed
    back to columns with K=1 matmuls against a 1x1 ones operand; the
    weighted sum is a K=128(s) M=1 matmul over the natural-layout gathered
    rows (pad rows are zeroed by the compact mask, so they add 0).
All matmuls run in bf16 with fp32 PSUM accumulation (measured end-to-end
scale-relative error ~2e-3 vs the fp32 reference).
"""

import numpy as np
from contextlib import ExitStack

B, S, ENC, DEC = 32, 2048, 1024, 1024
N_CORES = 8
BC = B // N_CORES  # batch rows per core
# padded compact-token count: Binomial(2048, 0.5) is 1024 +- 22.6, so 1152
# is a +5.7 sigma bound on the per-row unmasked count (~1e-8 per row;
# seed-0 data maxes at 1062)
P_PAD = 1152

# The sparse (mask-compaction) path is numerically validated on hardware
# (rel err 2.44e-3, identical to dense) using the HW-correct row-granularity
# scatter (one row index per partition, 16-byte payloads; elementwise and
# multi-index-per-partition scatters scramble on silicon). It cuts TensorE
# work ~36%, but the index build needs ~20 small SWDGE ops per batch row and
# the Q7 descriptor-generation rate (~1-3us per indirect op, serial) makes
# the whole pipeline Pool-bound: cost model 500us vs 352us dense. Dense
# ships; flip SPARSE=True to use the compaction path.
SPARSE = False


def build_bass_kernel(
    bc=BC, s=S, e_dim=ENC, d_dim=DEC, debug=False, sparse=SPARSE, p_pad=None
):
    import concourse.bass as bass
    import concourse.tile as tile
    from concourse import bacc, mybir

    f32 = mybir.dt.float32
    bf16 = mybir.dt.bfloat16
    i32 = mybir.dt.int32
    Tanh = mybir.ActivationFunctionType.Tanh
    Exp = mybir.ActivationFunctionType.Exp
    Alu = mybir.AluOpType

    assert s % 512 == 0 and e_dim % 512 == 0 and d_dim % 128 == 0
    if p_pad is None:
        p_pad = P_PAD if s == 2048 else (s // 2 + 128)
    if not sparse:
        p_pad = s
    assert p_pad % 128 == 0
    n_ct = p_pad // 128            # compact s-tiles per batch row
    # chunk sizes (matmul free dim), each <=512 and a multiple of 128
    chunk_sizes = []
    rem = p_pad
    while rem > 0:
        c = min(512, rem)
        chunk_sizes.append(c)
        rem -= c
    n_chunks = len(chunk_sizes)
    n_et = e_dim // 128            # contraction tiles for e_proj
    n_dt = d_dim // 128            # d (output) tiles for e_proj
    n_ec = e_dim // 512            # 512-wide e chunks for the weighted sum
    # (chunk, within-chunk) of each compact s-tile
    tile_map = []
    for c, csz in enumerate(chunk_sizes):
        for j in range(csz // 128):
            tile_map.append((c, j))

    nc = bacc.Bacc("TRN2", target_bir_lowering=False, debug=debug)

    hs_h = nc.dram_tensor("hidden_states", [bc, d_dim], f32, kind="ExternalInput")
    enc_h = nc.dram_tensor("encoder_outputs", [bc, s, e_dim], f32, kind="ExternalInput")
    msk_h = nc.dram_tensor("encoder_masks", [bc, s], i32, kind="ExternalInput")
    aw_h = nc.dram_tensor("a_w", [e_dim + d_dim, d_dim], f32, kind="ExternalInput")
    ab_h = nc.dram_tensor("a_b", [d_dim], f32, kind="ExternalInput")
    vw_h = nc.dram_tensor("v_w", [d_dim], f32, kind="ExternalInput")
    id_h = nc.dram_tensor("ident", [bc, bc], bf16, kind="ExternalInput")
    if sparse:
        iota_pf_h = nc.dram_tensor("iota_pf", [1, p_pad], f32, kind="ExternalInput")
        tokrep_h = nc.dram_tensor("tokrep", [128, s // 128, 4], i32, kind="ExternalInput")
        iota_ppi_h = nc.dram_tensor("iota_ppi", [128, p_pad // 128], i32, kind="ExternalInput")
    out_h = nc.dram_tensor("out", [bc, e_dim], f32, kind="ExternalOutput")

    enc_flat = enc_h[:, :, :].rearrange("b s e -> (b s) e")

    with tile.TileContext(nc) as tc, ExitStack() as ctx:
        consts = ctx.enter_context(tc.tile_pool(name="consts", bufs=1))
        enc_pool = ctx.enter_context(tc.tile_pool(name="enc", bufs=3 * n_chunks - 1 if sparse else 2 * n_chunks + 2))
        encT_pool = ctx.enter_context(tc.tile_pool(name="encT", bufs=2))
        tanh_pool = ctx.enter_context(tc.tile_pool(name="tanh", bufs=3))
        sm_pool = ctx.enter_context(tc.tile_pool(name="softmax", bufs=2))
        msk_pool = ctx.enter_context(tc.tile_pool(name="mask", bufs=2))
        small_pool = ctx.enter_context(tc.tile_pool(name="small", bufs=4))
        outsb_pool = ctx.enter_context(tc.tile_pool(name="outsb", bufs=1 if sparse else 2))
        pe_psum = ctx.enter_context(tc.tile_pool(name="pe_psum", bufs=2, space="PSUM"))
        sc_psum = ctx.enter_context(tc.tile_pool(name="sc_psum", bufs=2, space="PSUM"))
        at_psum = ctx.enter_context(tc.tile_pool(name="at_psum", bufs=1, space="PSUM"))
        w_psum = ctx.enter_context(tc.tile_pool(name="w_psum", bufs=2, space="PSUM"))
        if sparse:
            dram_pool = ctx.enter_context(
                tc.tile_pool(name="dram", bufs=2, space="DRAM")
            )

        # ---------------- prep: small tensors ----------------
        ident_sb = consts.tile([bc, bc], bf16)
        nc.sync.dma_start(out=ident_sb, in_=id_h[:, :])
        ones_bf = ident_sb[0:1, 0:1]

        hs_bf = consts.tile([bc, d_dim], bf16)
        nc.gpsimd.dma_start(out=hs_bf, in_=hs_h[:, :])  # cast f32->bf16

        v_sb = consts.tile([128, n_dt], bf16)
        nc.gpsimd.dma_start(out=v_sb, in_=vw_h[:].rearrange("(i p) -> p i", p=128))

        ab_sb = consts.tile([128, n_dt], f32)
        nc.sync.dma_start(out=ab_sb, in_=ab_h[:].rearrange("(i p) -> p i", p=128))

        if sparse:
            zeros_f = consts.tile([1, s], f32)
            nc.vector.memset(zeros_f, 0.0)
            iota_cf = consts.tile([1, p_pad], f32)
            nc.sync.dma_start(out=iota_cf, in_=iota_pf_h[:, :])
            tokrep_sb = consts.tile([128, s // 128, 4], i32)
            nc.sync.dma_start(out=tokrep_sb, in_=tokrep_h[:, :, :])
            iota_ppi = consts.tile([128, p_pad // 128], i32)
            nc.sync.dma_start(out=iota_ppi, in_=iota_ppi_h[:, :])
            zeros4 = consts.tile([128, 4], i32)
            nc.vector.memset(zeros4, 0)
            # two alternating DRAM index buffers (4-wide i32 rows; only
            # col 0 is consumed). Zero-init rows 0..p_pad-1 ONCE with the
            # HW-validated scatter shape: one row index per partition,
            # 16-byte row payload. Later batches overwrite the first
            # `count` rows; stale pad rows still hold valid (masked-out)
            # token ids.
            idx_bufs = []
            for nm in ("idxA", "idxB"):
                buf = dram_pool.tile([s, 4], i32, tag=nm)
                for j in range(p_pad // 128):
                    nc.gpsimd.indirect_dma_start(
                        out=buf[:, :],
                        out_offset=bass.IndirectOffsetOnAxis(
                            ap=iota_ppi[:, j : j + 1], axis=0
                        ),
                        in_=zeros4,
                        in_offset=None,
                    )
                idx_bufs.append(buf)

        state = {}

        def emit_loads(b):
            chunks = []
            if sparse:
                # ---- on-device compaction of unmasked token indices ----
                msk_b = msk_pool.tile([1, s], i32, tag="mask")
                nc.sync.dma_start(out=msk_b, in_=msk_h[b : b + 1, :])
                maskf = msk_pool.tile([1, s], f32, tag="maskf")
                nc.vector.tensor_copy(out=maskf, in_=msk_b)
                # inclusive prefix sum of the 0/1 mask
                cums = msk_pool.tile([1, s], f32, tag="cums")
                nc.vector.tensor_tensor_scan(
                    cums, maskf, zeros_f, 0.0, op0=Alu.add, op1=Alu.add
                )
                # compact-lane validity mask (count = last prefix value)
                count_ap = cums[0:1, s - 1 : s]
                maskc = sm_pool.tile([1, p_pad], bf16, tag="maskc")
                nc.vector.tensor_scalar(
                    maskc, iota_cf, count_ap, None, op0=Alu.is_lt
                )
                # compact position for kept tokens, dump row p_pad for
                # masked ones (collisions there are never read):
                # offi = (cums - (1 + p_pad)) * maskf + p_pad
                # (in-place into maskf, then int-cast into cums' bytes --
                # SBUF is tight with two batches of lookahead)
                nc.vector.scalar_tensor_tensor(
                    maskf, cums, -(1.0 + p_pad), maskf, op0=Alu.add, op1=Alu.mult
                )
                offi = cums.bitcast(i32)
                nc.vector.tensor_scalar(
                    offi, maskf, float(p_pad), None, op0=Alu.add
                )
                # round-trip through DRAM to get offsets in (partition, j)
                # layout: the HW scatter wants one row index per partition
                off_d = dram_pool.tile([1, s], i32, tag="offd")
                nc.sync.dma_start(out=off_d, in_=offi)
                offi_pb = msk_pool.tile([128, s // 128], i32, tag="offpb")
                nc.sync.dma_start(
                    out=offi_pb,
                    in_=off_d[0:1, :].rearrange("one (j p) -> p (j one)", p=128),
                )
                # global token ids for this batch row as 16-byte row payloads
                valb = msk_pool.tile([128, s // 128, 4], i32, tag="valb")
                nc.vector.tensor_scalar_add(valb, tokrep_sb, float(b * s))
                idx_d = idx_bufs[b % 2]
                for j in range(s // 128):
                    nc.gpsimd.indirect_dma_start(
                        out=idx_d[:, :],
                        out_offset=bass.IndirectOffsetOnAxis(
                            ap=offi_pb[:, j : j + 1], axis=0
                        ),
                        in_=valb[:, j, :],
                        in_offset=None,
                    )
                idx_sb = msk_pool.tile([128, n_ct, 4], i32, tag="idx_sb")
                nc.sync.dma_start(
                    out=idx_sb,
                    in_=idx_d[0:p_pad, :].rearrange("(j p) r -> p j r", p=128),
                )
                # gather unmasked encoder rows (cast f32->bf16 in the
                # DMA); one (128,1)-index call per compact s-tile — the
                # HW-validated gather shape
                g = 0
                for c, csz in enumerate(chunk_sizes):
                    st_c = csz // 128
                    enc_c = enc_pool.tile([128, 4, e_dim], bf16, tag="enc")
                    for jj in range(st_c):
                        nc.gpsimd.indirect_dma_start(
                            out=enc_c[:, jj, :],
                            out_offset=None,
                            in_=enc_flat,
                            in_offset=bass.IndirectOffsetOnAxis(
                                ap=idx_sb[:, g, 0:1], axis=0
                            ),
                        )
                        g += 1
                    chunks.append(enc_c)
                state[b] = dict(enc=chunks, pmask=maskc)
            else:
                pos = 0
                for t, csz in enumerate(chunk_sizes):
                    if b == 0 and t == 0:
                        chunks.append(enc_b0_c0)
                        pos += csz
                        continue
                    enc_c = enc_pool.tile([128, 4, e_dim], bf16, tag="enc")
                    nc.gpsimd.dma_start(
                        out=enc_c[:, 0 : csz // 128, :],
                        in_=enc_h[b, pos : pos + csz, :].rearrange(
                            "(j p) e -> p j e", p=128
                        ),
                    )
                    pos += csz
                    chunks.append(enc_c)
                msk_b = msk_pool.tile([1, s], i32, tag="mask")
                nc.sync.dma_start(out=msk_b, in_=msk_h[b : b + 1, :])
                maskf = msk_pool.tile([1, s], bf16, tag="maskf")
                nc.gpsimd.tensor_copy(out=maskf, in_=msk_b)
                state[b] = dict(enc=chunks, pmask=maskf)

        def emit_eproj_scores(b, mid_hook=None):
            chunks = state[b]["enc"]
            scores = sm_pool.tile([1, p_pad], f32, tag="scores")
            pos = 0
            for t, csz in enumerate(chunk_sizes):
                st_c = csz // 128
                if b == 0 and t == 0 and pre_encT is not None:
                    encT = pre_encT
                else:
                    encT = encT_pool.tile([128, n_et, 512], bf16, tag="encT")
                    for j in range(st_c):
                        nc.sync.dma_start(
                            out=encT[:, :, 128 * j : 128 * (j + 1)],
                            in_=chunks[t][:, j, :],
                            transpose=True,
                        )
                psum_sc = sc_psum.tile([1, csz], f32, tag="sc")
                for i in range(n_dt):
                    psum_e = pe_psum.tile([128, csz], f32, tag="pe")
                    for e in range(n_et):
                        nc.tensor.matmul(
                            psum_e,
                            lhsT=w_enc_sb[:, e, 128 * i : 128 * (i + 1)],
                            rhs=encT[:, e, 0:csz],
                            start=(e == 0),
                            stop=(e == n_et - 1),
                        )
                    if mid_hook is not None:
                        # h_proj/hb must be emitted before the first tanh
                        # that reads hb_sb (program-order RAW tracking), but
                        # after d0's matmuls so PE has work while w_dec lands
                        mid_hook()
                        mid_hook = None
                    th = tanh_pool.tile([128, csz], bf16, tag="tanh")
                    nc.scalar.activation(
                        th, psum_e, Tanh, bias=hb_sb[:, i, b : b + 1], scale=1.0
                    )
                    nc.tensor.matmul(
                        psum_sc,
                        lhsT=v_sb[:, i : i + 1],
                        rhs=th,
                        start=(i == 0),
                        stop=(i == n_dt - 1),
                    )
                nc.scalar.copy(scores[:, pos : pos + csz], psum_sc)
                pos += csz
            state[b]["scores"] = scores

        def emit_softmax(b):
            scores = state[b]["scores"]
            pmask = state[b]["pmask"]
            # no max-shift needed: |score| <= sum|v_d| = 32 strictly
            # (|tanh|<=1, |v_w|<=1/32), so exp cannot overflow fp32
            nc.scalar.activation(scores, scores, Exp, bias=0.0, scale=1.0)
            nc.vector.tensor_mul(scores, scores, pmask)
            ssum = small_pool.tile([1, 1], f32, tag="ssum")
            nc.vector.reduce_sum(out=ssum, in_=scores, axis=mybir.AxisListType.X)
            rsum = small_pool.tile([1, 1], f32, tag="rsum")
            nc.vector.reciprocal(rsum, ssum)
            attn_bf = sm_pool.tile([1, p_pad], bf16, tag="attn")
            nc.vector.tensor_scalar_mul(attn_bf, scores, rsum[0:1, 0:1])
            state[b]["attn"] = attn_bf

        def emit_attnT_weighted(b):
            chunks = state[b]["enc"]
            attn_bf = state[b]["attn"]
            # transpose attn row into columns: K=1 matmul against ones(1,1)
            psum_at = at_psum.tile([128, n_ct], f32, tag="at")
            for j in range(n_ct):
                nc.tensor.matmul(
                    psum_at[:, j : j + 1],
                    lhsT=attn_bf[:, 128 * j : 128 * (j + 1)],
                    rhs=ones_bf,
                    start=True,
                    stop=True,
                )
            attnT = small_pool.tile([128, n_ct], bf16, tag="attnT")
            nc.scalar.copy(attnT, psum_at)

            out_sb = outsb_pool.tile([1, e_dim], f32, tag="outsb")
            for ec in range(n_ec):
                psum_w = w_psum.tile([1, 512], f32, tag="w")
                for j in range(n_ct):
                    c, jj = tile_map[j]
                    nc.tensor.matmul(
                        psum_w,
                        lhsT=attnT[:, j : j + 1],
                        rhs=chunks[c][:, jj, 512 * ec : 512 * (ec + 1)],
                        start=(j == 0),
                        stop=(j == n_ct - 1),
                    )
                nc.scalar.copy(out_sb[:, 512 * ec : 512 * (ec + 1)], psum_w)
            nc.sync.dma_start(out=out_h[b : b + 1, :], in_=out_sb)

        # sparse: batch-0's index build + gathers overlap the weight DMA.
        # dense: batch-0 chunk 0 loads first, then w_enc (e_proj's weights),
        # then w_dec — so the first e_proj matmuls start ~12us in and the
        # tiny h_proj fills the remaining DMA latency
        if sparse:
            emit_loads(0)
        w_enc_sb = consts.tile([128, n_et, d_dim], bf16)
        nc.gpsimd.dma_start(
            out=w_enc_sb, in_=aw_h[d_dim:, :].rearrange("(k p) d -> p k d", p=128)
        )
        pre_encT = None
        if not sparse:
            enc_b0_c0 = enc_pool.tile([128, 4, e_dim], bf16, tag="enc")
            nc.gpsimd.dma_start(
                out=enc_b0_c0[:, 0 : chunk_sizes[0] // 128, :],
                in_=enc_h[0, 0 : chunk_sizes[0], :].rearrange(
                    "(j p) e -> p j e", p=128
                ),
            )
        wd_sb = consts.tile([128, n_dt, d_dim], bf16)
        nc.gpsimd.dma_start(
            out=wd_sb, in_=aw_h[0:d_dim, :].rearrange("(k p) d -> p k d", p=128)
        )

        hb_sb = consts.tile([128, n_dt, bc], f32)

        def emit_hproj():
            # hiddenT (d on partitions) via K=bc transpose-by-matmul.
            # PSUM->SBUF copies ride VectorE so they can't head-of-line
            # block the tanh ops already queued on ScalarE.
            psum_h = pe_psum.tile([128, n_dt * bc], f32, tag="pe")
            for k in range(n_dt):
                nc.tensor.matmul(
                    psum_h[:, bc * k : bc * (k + 1)],
                    lhsT=hs_bf[:, 128 * k : 128 * (k + 1)],
                    rhs=ident_sb,
                    start=True,
                    stop=True,
                )
            hT_sb = consts.tile([128, n_dt, bc], bf16)
            nc.vector.tensor_copy(hT_sb, psum_h)

            # h_projT[d, b] accumulated over dec-in tiles. One PSUM group
            # per (k, i) — PSUM start=True arms pending-zero for the whole
            # 2 KiB region, so cross-k accumulation happens in SBUF.
            hacc = consts.tile([128, n_dt * bc], f32)
            for k in range(n_dt):
                psum_hp = pe_psum.tile([128, n_dt * bc], f32, tag="pe")
                for i in range(n_dt):
                    nc.tensor.matmul(
                        psum_hp[:, bc * i : bc * (i + 1)],
                        lhsT=wd_sb[:, k, 128 * i : 128 * (i + 1)],
                        rhs=hT_sb[:, k, :],
                        start=True,
                        stop=True,
                    )
                if k == 0:
                    nc.vector.tensor_copy(hacc, psum_hp)
                else:
                    nc.vector.tensor_add(hacc, hacc, psum_hp)
            # hb[d, b] = h_projT + a_b  (per-partition bias for the tanh)
            for i in range(n_dt):
                nc.vector.tensor_scalar_add(
                    hb_sb[:, i, :], hacc[:, bc * i : bc * (i + 1)], ab_sb[:, i : i + 1]
                )

        if sparse:
            emit_hproj()
        if not sparse:
            emit_loads(0)
        if sparse and bc > 1:
            # two batches of load lookahead: the per-batch index-build +
            # scatter chain is ~Pool-bound and needs a head start
            emit_loads(1)

        # interleave so PE never waits on a softmax: weighted(b-1) runs
        # while softmax(b) is still on VectorE/ScalarE. attnT/weighted are
        # emitted BEFORE softmax(b) so their semaphore waits can't get
        # coarsened into waiting on batch b's softmax ops.
        for b in range(bc):
            if b > 0 and not (sparse and b == 1):
                emit_loads(b)
            emit_eproj_scores(
                b, mid_hook=emit_hproj if (b == 0 and not sparse) else None
            )
            if b >= 1:
                emit_attnT_weighted(b - 1)
            emit_softmax(b)
        emit_attnT_weighted(bc - 1)

    nc.compile()
    return nc


_CACHE = {}


def kernel(hidden_states, encoder_outputs, encoder_masks, a_w, a_b, v_w):
    import ml_dtypes
    from concourse.bass_utils import run_bass_kernel_spmd

    if "nc" not in _CACHE:
        _CACHE["nc"] = build_bass_kernel()
    nc = _CACHE["nc"]

    hidden_states = np.asarray(hidden_states, dtype=np.float32)
    encoder_outputs = np.asarray(encoder_outputs, dtype=np.float32)
    encoder_masks = np.asarray(encoder_masks, dtype=np.int32)
    a_w = np.ascontiguousarray(np.asarray(a_w, dtype=np.float32))
    a_b = np.ascontiguousarray(np.asarray(a_b, dtype=np.float32))
    v_w = np.ascontiguousarray(np.asarray(v_w, dtype=np.float32))
    ident = np.eye(BC, dtype=ml_dtypes.bfloat16)

    in_maps = []
    for c in range(N_CORES):
        sl = slice(c * BC, (c + 1) * BC)
        m = {
            "hidden_states": np.ascontiguousarray(hidden_states[sl]),
            "encoder_outputs": np.ascontiguousarray(encoder_outputs[sl]),
            "encoder_masks": np.ascontiguousarray(encoder_masks[sl]),
            "a_w": a_w,
            "a_b": a_b,
            "v_w": v_w,
            "ident": ident,
        }
        if SPARSE:
            m["iota_pf"] = np.arange(P_PAD, dtype=np.float32).reshape(1, P_PAD)
            tok = (
                np.arange(S // 128)[None, :] * 128 + np.arange(128)[:, None]
            ).astype(np.int32)
            m["tokrep"] = np.repeat(tok[:, :, None], 4, axis=2).copy()
            m["iota_ppi"] = np.ascontiguousarray(tok[:, : P_PAD // 128])
        in_maps.append(m)

    global _LAST_IN_MAPS
    _LAST_IN_MAPS = in_maps
    res = run_bass_kernel_spmd(nc, in_maps, core_ids=list(range(N_CORES)))
    out = np.concatenate([r["out"] for r in res.results], axis=0)
    return out.astype(np.float32)


_LAST_IN_MAPS = None



# revision 6
# speedup vs baseline: 1.4247x; 1.4247x over previous
"""Bahdanau-style attention kernel for Trainium2 (8 NeuronCores, SPMD).

Math (per batch row b):
    h_proj = hidden @ a_w[:DEC]                       (DEC,)
    e_proj[s, :] = enc[s, :] @ a_w[DEC:]              (S, DEC)
    energy = tanh(e_proj + h_proj + a_b)              (S, DEC)
    scores = energy @ v_w                             (S,)
    scores = where(mask == 0, -1e10, scores)
    attn = softmax(scores)                            (S,)
    out = attn @ enc                                  (ENC,)

Sharding: data-parallel over batch (32 rows -> 4 rows on each of 8 cores);
weights replicated (pre-quantized to fp8*64 on host).

Per-core strategy:
  - enc arrives as bf16 (host-cast); natural chunks [tok, e] feed the final
    weighted sum in bf16 (softmax-averaging keeps fp8 quantization error
    ~2.7% in the output, so the weighted sum must stay >= 16-bit).
  - e_proj runs in fp8 with MatmulPerfMode.DoubleRow (2 k-tiles per
    instruction at 0.5 cycles/row): encT fp8 tiles are produced by xbar
    DMA transpose (bf16) + engine cast, or PE is_transpose matmuls + cast
    evacuation for a tunable subset of chunks (balances DMA vs PE).
  - e_proj PSUM is [128d, 2x512tok] (a chunk pair, 2 banks) so one tanh
    activation covers 1024 tokens per d-tile, amortizing the ~185ns
    ScalarE access overhead; bias (h_proj + a_b) is per-partition.
  - scores = v . tanh as fp8 DoubleRow over d-tile pairs; exp is fused
    into the PSUM evacuation (activation Exp, scale=1/64 undoing the *64
    weight scaling). Softmax rest on DVE; weighted sum bf16 on PE.
"""

import numpy as np
from contextlib import ExitStack

B, S, ENC, DEC = 32, 2048, 1024, 1024
N_CORES = 8
BC = B // N_CORES  # batch rows per core
CH = 512           # tokens per chunk

# chunks whose transpose runs on the PE (is_transpose matmuls) instead of
# the xbar DMA: (b, c) pairs. Balances the DMA device against TensorE.
PE_CHUNKS = frozenset()
# engine for each chunk's bf16->fp8 cast (rotates): entries in {"v", "p", "s"}
CAST_ROTATION = ("p", "v", "p", "v")


def build_bass_kernel(
    bc=BC, s=S, e_dim=ENC, d_dim=DEC, debug=False,
    pe_chunks=PE_CHUNKS, cast_rotation=CAST_ROTATION,
):
    import concourse.bass as bass
    import concourse.tile as tile
    from concourse import bacc, mybir

    f32 = mybir.dt.float32
    bf16 = mybir.dt.bfloat16
    fp8 = mybir.dt.float8e4
    i32 = mybir.dt.int32
    Tanh = mybir.ActivationFunctionType.Tanh
    Exp = mybir.ActivationFunctionType.Exp
    Copy = mybir.ActivationFunctionType.Copy
    DR = mybir.MatmulPerfMode.DoubleRow

    assert s % (2 * CH) == 0 and e_dim % 256 == 0 and d_dim % 256 == 0
    n_chunks = s // CH             # 512-token chunks per batch row
    n_st = CH // 128               # s-tiles per chunk
    n_et = e_dim // 128            # contraction tiles for e_proj
    n_dt = d_dim // 128            # d (output) tiles for e_proj
    n_ec = e_dim // 512            # 512-wide e chunks for the weighted sum
    n_ct = s // 128                # s-tiles per row

    nc = bacc.Bacc("TRN2", target_bir_lowering=False, debug=debug)

    enc_h = nc.dram_tensor("enc_bf", [bc, s, e_dim], bf16, kind="ExternalInput")
    msk_h = nc.dram_tensor("encoder_masks", [bc, s], i32, kind="ExternalInput")
    w8_h = nc.dram_tensor("w8", [128, n_et, d_dim], fp8, kind="ExternalInput")
    wd8_h = nc.dram_tensor("wd8", [128, n_dt, d_dim], fp8, kind="ExternalInput")
    hsT8_h = nc.dram_tensor("hsT8", [128, n_dt, bc], fp8, kind="ExternalInput")
    ab_h = nc.dram_tensor("ab_t", [128, n_dt], f32, kind="ExternalInput")
    v8_h = nc.dram_tensor("v8", [128, n_dt, 2], fp8, kind="ExternalInput")
    id_h = nc.dram_tensor("ident", [128, 128], bf16, kind="ExternalInput")
    out_h = nc.dram_tensor("out", [bc, e_dim], f32, kind="ExternalOutput")

    with tile.TileContext(nc) as tc, ExitStack() as ctx:
        consts = ctx.enter_context(tc.tile_pool(name="consts", bufs=1))
        nat_pool = ctx.enter_context(tc.tile_pool(name="nat", bufs=n_chunks + 3))
        encTb_pool = ctx.enter_context(tc.tile_pool(name="encTb", bufs=2))
        encT8_pool = ctx.enter_context(tc.tile_pool(name="encT8", bufs=n_chunks + 3))
        th_pool = ctx.enter_context(tc.tile_pool(name="th", bufs=2))
        sm_pool = ctx.enter_context(tc.tile_pool(name="softmax", bufs=2))
        msk_pool = ctx.enter_context(tc.tile_pool(name="mask", bufs=2))
        small_pool = ctx.enter_context(tc.tile_pool(name="small", bufs=4))
        outsb_pool = ctx.enter_context(tc.tile_pool(name="outsb", bufs=2))
        pe_psum = ctx.enter_context(tc.tile_pool(name="pe_psum", bufs=2, space="PSUM"))
        sc_psum = ctx.enter_context(tc.tile_pool(name="sc_psum", bufs=1, space="PSUM"))
        w_psum = ctx.enter_context(tc.tile_pool(name="w_psum", bufs=1, space="PSUM"))
        misc_psum = ctx.enter_context(tc.tile_pool(name="misc_psum", bufs=2, space="PSUM"))

        # ---------------- consts ----------------
        ident_sb = consts.tile([128, 128], bf16)
        nc.sync.dma_start(out=ident_sb, in_=id_h[:, :])
        ones_bf = ident_sb[0:1, 0:1]

        w8_sb = consts.tile([128, n_et, d_dim], fp8)
        nc.sync.dma_start(out=w8_sb, in_=w8_h[:, :, :])
        # dual-fp8 ldweights needs a wide stride between the k-pair weight
        # blocks (walrus s3_lw_dual_fp8_restrictions rejects stride 2/4;
        # 512 verified on HW) -> stage v into a padded tile
        v8_sb = consts.tile([128, n_dt, 512], fp8)
        nc.sync.dma_start(out=v8_sb[:, :, 0:2], in_=v8_h[:, :, :])
        ab_sb = consts.tile([128, n_dt], f32)
        nc.sync.dma_start(out=ab_sb, in_=ab_h[:, :])
        hsT8_sb = consts.tile([128, n_dt, bc], fp8)
        nc.sync.dma_start(out=hsT8_sb, in_=hsT8_h[:, :, :])
        wd8_sb = consts.tile([128, n_dt, d_dim], fp8)
        nc.sync.dma_start(out=wd8_sb, in_=wd8_h[:, :, :])

        hb_sb = consts.tile([128, n_dt, bc], f32)

        state = {}

        def emit_load_chunk(b, c):
            nat = nat_pool.tile([128, n_st, e_dim], bf16, tag="nat")
            nc.sync.dma_start(
                out=nat,
                in_=enc_h[b, CH * c : CH * (c + 1), :].rearrange(
                    "(j p) e -> p j e", p=128
                ),
            )
            state[(b, c)] = dict(nat=nat)

        def emit_transpose_chunk(b, c):
            nat = state[(b, c)]["nat"]
            encT8 = encT8_pool.tile([128, n_et, CH], fp8, tag="encT8")
            eng = cast_rotation[c % len(cast_rotation)]
            if (b, c) in pe_chunks:
                # PE path: per s-tile j, transpose all e-tiles into one psum
                # bank, evacuate with a cast into the strided encT8 columns
                for j in range(n_st):
                    pt = misc_psum.tile([128, n_et, 128], bf16, tag="misc")
                    for u in range(n_et):
                        nc.tensor.matmul(
                            pt[:, u, :],
                            lhsT=nat[:, j, 128 * u : 128 * (u + 1)],
                            rhs=ident_sb,
                            start=True,
                            stop=True,
                            is_transpose=True,
                        )
                    dst = encT8[:, :, 128 * j : 128 * (j + 1)]
                    if eng == "v":
                        nc.vector.tensor_copy(out=dst, in_=pt)
                    else:
                        nc.scalar.activation(dst, pt, Copy, bias=0.0, scale=1.0)
            else:
                encTb = encTb_pool.tile([128, n_et, CH], bf16, tag="encTb")
                for j in range(n_st):
                    nc.sync.dma_start(
                        out=encTb[:, :, 128 * j : 128 * (j + 1)],
                        in_=nat[:, j, :],
                        transpose=True,
                    )
                if eng == "p":
                    nc.gpsimd.tensor_copy(out=encT8, in_=encTb)
                elif eng == "v":
                    nc.vector.tensor_copy(out=encT8, in_=encTb)
                else:
                    nc.scalar.activation(encT8, encTb, Copy, bias=0.0, scale=1.0)
            state[(b, c)]["encT8"] = encT8

        def emit_hproj():
            hp = misc_psum.tile([128, n_dt, bc], f32, tag="misc")
            for i in range(n_dt):
                for u in range(n_dt // 2):
                    nc.tensor.matmul(
                        hp[:, i, :],
                        lhsT=wd8_sb[:, 2 * u : 2 * u + 2, 128 * i : 128 * (i + 1)],
                        rhs=hsT8_sb[:, 2 * u : 2 * u + 2, :],
                        start=(u == 0),
                        stop=(u == n_dt // 2 - 1),
                        perf_mode=DR,
                    )
            for i in range(n_dt):
                nc.vector.tensor_scalar(
                    hb_sb[:, i, :], hp[:, i, :], 1.0 / 64,
                    ab_sb[:, i : i + 1],
                    op0=mybir.AluOpType.mult, op1=mybir.AluOpType.add,
                )

        def emit_eproj_pair(b, cp):
            c0, c1 = 2 * cp, 2 * cp + 1
            eT = (state[(b, c0)]["encT8"], state[(b, c1)]["encT8"])
            scores = state[b]["scores"]
            th = th_pool.tile([128, n_dt, 2 * CH], fp8, tag="th")
            for i in range(n_dt):
                ps = pe_psum.tile([128, 2, CH], f32, tag="pe")
                for h in range(2):
                    for u in range(n_et // 2):
                        nc.tensor.matmul(
                            ps[:, h, :],
                            lhsT=w8_sb[:, 2 * u : 2 * u + 2, 128 * i : 128 * (i + 1)],
                            rhs=eT[h][:, 2 * u : 2 * u + 2, :],
                            start=(u == 0),
                            stop=(u == n_et // 2 - 1),
                            perf_mode=DR,
                        )
                nc.scalar.activation(
                    th[:, i, :], ps, Tanh, bias=hb_sb[:, i, b : b + 1], scale=1.0 / 64
                )
            for h in range(2):
                sc = sc_psum.tile([2, CH], f32, tag="sc")
                for m in range(n_dt // 2):
                    nc.tensor.matmul(
                        sc,
                        lhsT=v8_sb[:, 2 * m : 2 * m + 2, 0:2],
                        rhs=th[:, 2 * m : 2 * m + 2, CH * h : CH * (h + 1)],
                        start=(m == 0),
                        stop=(m == n_dt // 2 - 1),
                        perf_mode=DR,
                    )
                pos = CH * (2 * cp + h)
                nc.scalar.activation(
                    scores[:, pos : pos + CH], sc[0:1, :], Exp, bias=0.0, scale=1.0 / 64
                )

        def emit_row_prep(b):
            msk_b = msk_pool.tile([1, s], i32, tag="mask")
            nc.sync.dma_start(out=msk_b, in_=msk_h[b : b + 1, :])
            maskf = msk_pool.tile([1, s], bf16, tag="maskf")
            nc.gpsimd.tensor_copy(out=maskf, in_=msk_b)
            scores = sm_pool.tile([1, s], f32, tag="scores")
            state[b] = dict(pmask=maskf, scores=scores)

        def emit_softmax(b):
            # scores already hold exp(score); mask, normalize. No max-shift
            # needed: |score| <= sum|v_d| = 32 so exp stays in fp32 range.
            scores = state[b]["scores"]
            nc.vector.tensor_mul(scores, scores, state[b]["pmask"])
            ssum = small_pool.tile([1, 1], f32, tag="ssum")
            nc.vector.reduce_sum(out=ssum, in_=scores, axis=mybir.AxisListType.X)
            rsum = small_pool.tile([1, 1], f32, tag="rsum")
            nc.vector.reciprocal(rsum, ssum)
            attn_bf = sm_pool.tile([1, s], bf16, tag="attn")
            nc.vector.tensor_scalar_mul(attn_bf, scores, rsum[0:1, 0:1])
            state[b]["attn"] = attn_bf

        def emit_attnT_weighted(b):
            attn_bf = state[b]["attn"]
            psum_at = misc_psum.tile([128, n_ct], f32, tag="misc")
            for j in range(n_ct):
                nc.tensor.matmul(
                    psum_at[:, j : j + 1],
                    lhsT=attn_bf[:, 128 * j : 128 * (j + 1)],
                    rhs=ones_bf,
                    start=True,
                    stop=True,
                )
            attnT = small_pool.tile([128, n_ct], bf16, tag="attnT")
            nc.scalar.copy(attnT, psum_at)

            out_sb = outsb_pool.tile([1, e_dim], f32, tag="outsb")
            for ec in range(n_ec):
                pw = w_psum.tile([1, 512], f32, tag="w")
                for j in range(n_ct):
                    c, jj = divmod(j, n_st)
                    nc.tensor.matmul(
                        pw,
                        lhsT=attnT[:, j : j + 1],
                        rhs=state[(b, c)]["nat"][:, jj, 512 * ec : 512 * (ec + 1)],
                        start=(j == 0),
                        stop=(j == n_ct - 1),
                    )
                nc.scalar.copy(out_sb[:, 512 * ec : 512 * (ec + 1)], pw)
            nc.sync.dma_start(out=out_h[b : b + 1, :], in_=out_sb)

        # ---------------- schedule ----------------
        emit_row_prep(0)
        for c in range(n_chunks):
            emit_load_chunk(0, c)
            emit_transpose_chunk(0, c)
        emit_hproj()
        for b in range(bc):
            if b + 1 < bc:
                emit_row_prep(b + 1)
                for c in range(n_chunks):
                    emit_load_chunk(b + 1, c)
                    emit_transpose_chunk(b + 1, c)
            for cp in range(n_chunks // 2):
                emit_eproj_pair(b, cp)
            if b >= 1:
                emit_attnT_weighted(b - 1)
            emit_softmax(b)
        emit_attnT_weighted(bc - 1)

    nc.compile()
    return nc


_CACHE = {}


def _prep_weights(a_w, a_b, v_w, e_dim=ENC, d_dim=DEC):
    import ml_dtypes

    fp8 = ml_dtypes.float8_e4m3
    n_et, n_dt = e_dim // 128, d_dim // 128
    w8 = (
        (np.asarray(a_w[d_dim:], np.float32) * 64.0)
        .reshape(n_et, 128, d_dim).transpose(1, 0, 2).astype(fp8)
    )
    wd8 = (
        (np.asarray(a_w[:d_dim], np.float32) * 64.0)
        .reshape(n_dt, 128, d_dim).transpose(1, 0, 2).astype(fp8)
    )
    v8 = np.repeat(
        (np.asarray(v_w, np.float32) * 64.0)
        .reshape(n_dt, 128).T.reshape(128, n_dt, 1).astype(fp8),
        2, axis=2,
    )
    ab_t = np.ascontiguousarray(
        np.asarray(a_b, np.float32).reshape(n_dt, 128).T
    )
    return (
        np.ascontiguousarray(w8),
        np.ascontiguousarray(wd8),
        np.ascontiguousarray(v8),
        ab_t,
    )


def kernel(hidden_states, encoder_outputs, encoder_masks, a_w, a_b, v_w):
    import ml_dtypes
    from concourse.bass_utils import run_bass_kernel_spmd

    if "nc" not in _CACHE:
        _CACHE["nc"] = build_bass_kernel()
    nc = _CACHE["nc"]

    bf16 = ml_dtypes.bfloat16
    fp8 = ml_dtypes.float8_e4m3
    hidden_states = np.asarray(hidden_states, dtype=np.float32)
    enc_bf = np.asarray(encoder_outputs, dtype=np.float32).astype(bf16)
    encoder_masks = np.asarray(encoder_masks, dtype=np.int32)
    w8, wd8, v8, ab_t = _prep_weights(a_w, a_b, v_w)
    ident = np.eye(128, dtype=bf16)
    n_dt = DEC // 128

    in_maps = []
    for c in range(N_CORES):
        sl = slice(c * BC, (c + 1) * BC)
        hsT8 = np.ascontiguousarray(
            hidden_states[sl].T.reshape(n_dt, 128, BC).transpose(1, 0, 2)
        ).astype(fp8)
        m = {
            "enc_bf": np.ascontiguousarray(enc_bf[sl]),
            "encoder_masks": np.ascontiguousarray(encoder_masks[sl]),
            "w8": w8,
            "wd8": wd8,
            "hsT8": np.ascontiguousarray(hsT8),
            "ab_t": ab_t,
            "v8": v8,
            "ident": ident,
        }
        in_maps.append(m)

    global _LAST_IN_MAPS
    _LAST_IN_MAPS = in_maps
    res = run_bass_kernel_spmd(nc, in_maps, core_ids=list(range(N_CORES)))
    out = np.concatenate([r["out"] for r in res.results], axis=0)
    return out.astype(np.float32)


_LAST_IN_MAPS = None


# revision 10
# speedup vs baseline: 1.9746x; 1.3860x over previous
"""Bahdanau-style attention kernel for Trainium2 (8 NeuronCores, SPMD).

Math (per batch row b):
    h_proj = hidden @ a_w[:DEC]                       (DEC,)
    e_proj[s, :] = enc[s, :] @ a_w[DEC:]              (S, DEC)
    energy = tanh(e_proj + h_proj + a_b)              (S, DEC)
    scores = energy @ v_w                             (S,)
    scores = where(mask == 0, -1e10, scores)
    attn = softmax(scores)                            (S,)
    out = attn @ enc                                  (ENC,)

Sharding: data-parallel over batch (32 rows -> 4 rows on each of 8 cores);
weights replicated (pre-quantized to fp8*64 on host).

Per-core strategy:
  - The weighted sum runs in bf16 from natural-layout [tok, e] chunks
    (host-cast enc); softmax-averaging keeps per-element quantization
    error in the output, so fp8 enc there would blow the 2e-2 gate.
  - e_proj runs in fp8 with MatmulPerfMode.DoubleRow (2 k-tiles per
    instruction at 0.5 cycles/row). The transposed fp8 operand comes from
    the xbar DMA transpose moving fp8 PAIRS as uint16 lanes straight from
    DRAM: out[p, g, q](u16) = enc-pair(e=2(128g+p)(+0/1), tok q). The pair
    interleave is absorbed by the DoubleRow k-pair dimension with a
    host-permuted weight layout w8[p, g, i, d] = 64*a_w[DEC+2(128g+p)+i, d],
    so no on-chip bf16->fp8 cast and no bf16 staging is needed.
  - e_proj PSUM is [128d, 2x512tok] (a chunk pair, 2 banks) so one tanh
    activation covers 1024 tokens per d-tile, amortizing the ~185ns
    ScalarE access overhead; bias (h_proj + a_b) is per-partition.
  - scores = v . tanh as fp8 DoubleRow over d-tile pairs (v padded to
    M=2 / k-stride 512 for the dual-fp8 ldweights ISA restriction); exp
    is fused into the PSUM evacuation (scale=1/64 undoes the *64 weight
    scaling). Softmax tail on DVE.
  - The weighted sum accumulates chunk-major into one PSUM bank
    (e-halves at partitions 0/32) so nat buffers free chunk-by-chunk,
    and each iteration emits weighted(b-1) before eproj(b) so next-row
    DMA overlaps this row's PE work.
"""

import numpy as np
from contextlib import ExitStack

B, S, ENC, DEC = 32, 2048, 1024, 1024
N_CORES = 8
BC = B // N_CORES  # batch rows per core
CH = 512           # tokens per chunk


def build_bass_kernel(bc=BC, s=S, e_dim=ENC, d_dim=DEC, debug=False):
    import concourse.bass as bass
    import concourse.tile as tile
    from concourse import bacc, mybir

    f32 = mybir.dt.float32
    bf16 = mybir.dt.bfloat16
    fp8 = mybir.dt.float8e4
    u16 = mybir.dt.uint16
    i32 = mybir.dt.int32
    Tanh = mybir.ActivationFunctionType.Tanh
    Exp = mybir.ActivationFunctionType.Exp
    DR = mybir.MatmulPerfMode.DoubleRow

    assert s % (2 * CH) == 0 and e_dim % 256 == 0 and d_dim % 256 == 0
    n_chunks = s // CH             # 512-token chunks per batch row
    n_st = CH // 128               # s-tiles per chunk
    n_g = e_dim // 256             # e pair-groups (256 e-rows per group)
    n_dt = d_dim // 128            # d (output) tiles for e_proj
    n_ec = e_dim // 512            # 512-wide e chunks for the weighted sum
    n_ct = s // 128                # s-tiles per row

    nc = bacc.Bacc("TRN2", target_bir_lowering=False, debug=debug)

    enc_h = nc.dram_tensor("enc_bf", [bc, s, e_dim], bf16, kind="ExternalInput")
    enc8_h = nc.dram_tensor("enc8u", [bc, s, e_dim // 2], u16, kind="ExternalInput")
    msk_h = nc.dram_tensor("encoder_masks", [bc, s], i32, kind="ExternalInput")
    w8_h = nc.dram_tensor("w8", [128, n_g, 2, d_dim], fp8, kind="ExternalInput")
    wd8_h = nc.dram_tensor("wd8", [128, n_dt, d_dim], fp8, kind="ExternalInput")
    hsT8_h = nc.dram_tensor("hsT8", [128, n_dt, bc], fp8, kind="ExternalInput")
    ab_h = nc.dram_tensor("ab_t", [128, n_dt], f32, kind="ExternalInput")
    v8_h = nc.dram_tensor("v8", [128, n_dt, 2], fp8, kind="ExternalInput")
    id_h = nc.dram_tensor("ident", [128, 128], bf16, kind="ExternalInput")
    out_h = nc.dram_tensor("out", [bc, e_dim], f32, kind="ExternalOutput")

    with tile.TileContext(nc) as tc, ExitStack() as ctx:
        consts = ctx.enter_context(tc.tile_pool(name="consts", bufs=1))
        nat_pool = ctx.enter_context(tc.tile_pool(name="nat", bufs=2 * n_chunks))
        eT_pool = ctx.enter_context(tc.tile_pool(name="eT", bufs=2 * n_chunks + 1))
        th_pool = ctx.enter_context(tc.tile_pool(name="th", bufs=2))
        sm_pool = ctx.enter_context(tc.tile_pool(name="softmax", bufs=2))
        msk_pool = ctx.enter_context(tc.tile_pool(name="mask", bufs=2))
        small_pool = ctx.enter_context(tc.tile_pool(name="small", bufs=4))
        outsb_pool = ctx.enter_context(tc.tile_pool(name="outsb", bufs=1))
        pe_psum = ctx.enter_context(tc.tile_pool(name="pe_psum", bufs=2, space="PSUM"))
        sc_psum = ctx.enter_context(tc.tile_pool(name="sc_psum", bufs=1, space="PSUM"))
        w_psum = ctx.enter_context(tc.tile_pool(name="w_psum", bufs=1, space="PSUM"))
        misc_psum = ctx.enter_context(tc.tile_pool(name="misc_psum", bufs=2, space="PSUM"))

        # ---------------- consts ----------------
        ident_sb = consts.tile([128, 128], bf16)
        nc.sync.dma_start(out=ident_sb, in_=id_h[:, :])
        ones_bf = ident_sb[0:1, 0:1]

        w8_sb = consts.tile([128, n_g, 2, d_dim], fp8)
        # dual-fp8 ldweights needs a wide stride between the k-pair weight
        # blocks (walrus s3_lw_dual_fp8_restrictions rejects stride 2/4;
        # 512 verified on HW) -> stage v into a padded tile
        v8_sb = consts.tile([128, n_dt, 512], fp8)
        ab_sb = consts.tile([128, n_dt], f32)
        hsT8_sb = consts.tile([128, n_dt, bc], fp8)
        wd8_sb = consts.tile([128, n_dt, d_dim], fp8)

        def emit_consts(step):
            # spread const loads between row-0 chunk work so the first
            # e_proj matmuls aren't starved behind them on the DMA device
            if step == 0:
                nc.sync.dma_start(out=w8_sb, in_=w8_h[:, :, :, :])
            elif step == 1:
                nc.sync.dma_start(out=wd8_sb, in_=wd8_h[:, :, :])
                nc.sync.dma_start(out=hsT8_sb, in_=hsT8_h[:, :, :])
                nc.sync.dma_start(out=ab_sb, in_=ab_h[:, :])
            elif step == 2:
                nc.sync.dma_start(out=v8_sb[:, :, 0:2], in_=v8_h[:, :, :])

        hb_sb = consts.tile([128, n_dt, bc], f32)

        state = {}

        def emit_xbar_chunk(b, c):
            # transpose fp8 pairs (as u16 lanes) straight from DRAM:
            # out[p, j, g, q] = enc8u[b, CH*c + 128*j + q, 128*g + p]
            eT = eT_pool.tile([128, n_st, n_g, 128], u16, tag="eT")
            for j in range(n_st):
                nc.sync.dma_start(
                    out=eT[:, j, :, :],
                    in_=enc8_h[b, CH * c + 128 * j : CH * c + 128 * (j + 1), :],
                    transpose=True,
                )
            state[(b, c)] = dict(eT=eT)

        def emit_load_chunk(b, c):
            nat = nat_pool.tile([128, n_st, e_dim], bf16, tag="nat")
            nc.sync.dma_start(
                out=nat,
                in_=enc_h[b, CH * c : CH * (c + 1), :].rearrange(
                    "(j p) e -> p j e", p=128
                ),
            )
            state[(b, c)]["nat"] = nat

        def emit_hproj():
            hp = misc_psum.tile([128, n_dt, bc], f32, tag="misc")
            for i in range(n_dt):
                for u in range(n_dt // 2):
                    nc.tensor.matmul(
                        hp[:, i, :],
                        lhsT=wd8_sb[:, 2 * u : 2 * u + 2, 128 * i : 128 * (i + 1)],
                        rhs=hsT8_sb[:, 2 * u : 2 * u + 2, :],
                        start=(u == 0),
                        stop=(u == n_dt // 2 - 1),
                        perf_mode=DR,
                    )
            for i in range(n_dt):
                nc.vector.tensor_scalar(
                    hb_sb[:, i, :], hp[:, i, :], 1.0 / 64,
                    ab_sb[:, i : i + 1],
                    op0=mybir.AluOpType.mult, op1=mybir.AluOpType.add,
                )

        def emit_eproj_pair(b, cp):
            eT = []
            for c in (2 * cp, 2 * cp + 1):
                # [p, j, g, q](u16) -> fp8 [p, j, g, (q two)]; per (j, g) the
                # DoubleRow rhs is [p, two, q]
                eT.append(state[(b, c)]["eT"][:, :, :, :].bitcast(fp8))
            scores = state[b]["scores"]
            th = th_pool.tile([128, n_dt, 2 * CH], fp8, tag="th")
            for i in range(n_dt):
                ps = pe_psum.tile([128, 2, CH], f32, tag="pe")
                for h in range(2):
                    for j in range(n_st):
                        for g in range(n_g):
                            rhs = eT[h][:, j, g, :].rearrange(
                                "p (q two) -> p two q", two=2
                            )
                            nc.tensor.matmul(
                                ps[:, h, 128 * j : 128 * (j + 1)],
                                lhsT=w8_sb[:, g, :, 128 * i : 128 * (i + 1)],
                                rhs=rhs,
                                start=(g == 0),
                                stop=(g == n_g - 1),
                                perf_mode=DR,
                            )
                nc.scalar.activation(
                    th[:, i, :], ps, Tanh, bias=hb_sb[:, i, b : b + 1], scale=1.0 / 64
                )
            for h in range(2):
                sc = sc_psum.tile([2, CH], f32, tag="sc")
                for m in range(n_dt // 2):
                    nc.tensor.matmul(
                        sc,
                        lhsT=v8_sb[:, 2 * m : 2 * m + 2, 0:2],
                        rhs=th[:, 2 * m : 2 * m + 2, CH * h : CH * (h + 1)],
                        start=(m == 0),
                        stop=(m == n_dt // 2 - 1),
                        perf_mode=DR,
                    )
                pos = CH * (2 * cp + h)
                nc.scalar.activation(
                    scores[:, pos : pos + CH], sc[0:1, :], Exp, bias=0.0, scale=1.0 / 64
                )

        def emit_row_prep(b):
            msk_b = msk_pool.tile([1, s], i32, tag="mask")
            nc.sync.dma_start(out=msk_b, in_=msk_h[b : b + 1, :])
            maskf = msk_pool.tile([1, s], bf16, tag="maskf")
            nc.gpsimd.tensor_copy(out=maskf, in_=msk_b)
            scores = sm_pool.tile([1, s], f32, tag="scores")
            state[b] = dict(pmask=maskf, scores=scores)

        def emit_softmax(b):
            # scores already hold exp(score); mask, normalize. No max-shift
            # needed: |score| <= sum|v_d| = 32 so exp stays in fp32 range.
            scores = state[b]["scores"]
            nc.vector.tensor_mul(scores, scores, state[b]["pmask"])
            ssum = small_pool.tile([1, 1], f32, tag="ssum")
            nc.vector.reduce_sum(out=ssum, in_=scores, axis=mybir.AxisListType.X)
            rsum = small_pool.tile([1, 1], f32, tag="rsum")
            nc.vector.reciprocal(rsum, ssum)
            attn_bf = sm_pool.tile([1, s], bf16, tag="attn")
            nc.vector.tensor_scalar_mul(attn_bf, scores, rsum[0:1, 0:1])
            state[b]["attn"] = attn_bf

        def emit_attnT_weighted(b):
            attn_bf = state[b]["attn"]
            psum_at = misc_psum.tile([128, n_ct], f32, tag="misc")
            for j in range(n_ct):
                nc.tensor.matmul(
                    psum_at[:, j : j + 1],
                    lhsT=attn_bf[:, 128 * j : 128 * (j + 1)],
                    rhs=ones_bf,
                    start=True,
                    stop=True,
                )
            attnT = small_pool.tile([128, n_ct], bf16, tag="attnT")
            nc.scalar.copy(attnT, psum_at)

            out_sb = outsb_pool.tile([1, e_dim], f32, tag="outsb")
            # chunk-major so nat(b, c) is released after its own chunk's
            # matmuls; the n_ec accumulators live at partitions 0/32 of one
            # psum bank (out base partition must be a multiple of 32)
            pw = w_psum.tile([128, 512], f32, tag="w")
            for c in range(n_chunks):
                for ec in range(n_ec):
                    for jj in range(n_st):
                        nc.tensor.matmul(
                            pw[32 * ec : 32 * ec + 1, :],
                            lhsT=attnT[:, c * n_st + jj : c * n_st + jj + 1],
                            rhs=state[(b, c)]["nat"][:, jj, 512 * ec : 512 * (ec + 1)],
                            start=(c == 0 and jj == 0),
                            stop=(c == n_chunks - 1 and jj == n_st - 1),
                        )
            for ec in range(n_ec):
                nc.scalar.copy(
                    out_sb[:, 512 * ec : 512 * (ec + 1)], pw[32 * ec : 32 * ec + 1, :]
                )
            nc.sync.dma_start(out=out_h[b : b + 1, :], in_=out_sb)

        # ---------------- schedule ----------------
        emit_row_prep(0)
        for c in range(n_chunks):
            emit_xbar_chunk(0, c)
            emit_consts(c)
            emit_load_chunk(0, c)
        emit_hproj()
        for b in range(bc):
            # weighted(b-1) first: it frees row b-1's nat buffers chunk by
            # chunk, unblocking the row b+1 loads emitted right below while
            # the PE then chews on eproj(b)
            if b >= 1:
                emit_attnT_weighted(b - 1)
            if b + 1 < bc:
                emit_row_prep(b + 1)
                for c in range(n_chunks):
                    emit_xbar_chunk(b + 1, c)
                for c in range(n_chunks):
                    emit_load_chunk(b + 1, c)
            for cp in range(n_chunks // 2):
                emit_eproj_pair(b, cp)
            emit_softmax(b)
        emit_attnT_weighted(bc - 1)

    nc.compile()
    return nc


_CACHE = {}


def _prep_weights(a_w, a_b, v_w, e_dim=ENC, d_dim=DEC):
    import ml_dtypes

    fp8 = ml_dtypes.float8_e4m3
    n_g, n_dt = e_dim // 256, d_dim // 128
    # w8[p, g, i, d] = 64 * a_w[DEC + 2*(128*g + p) + i, d]
    w8 = (
        (np.asarray(a_w[d_dim:], np.float32) * 64.0)
        .reshape(n_g, 128, 2, d_dim).transpose(1, 0, 2, 3).astype(fp8)
    )
    wd8 = (
        (np.asarray(a_w[:d_dim], np.float32) * 64.0)
        .reshape(n_dt, 128, d_dim).transpose(1, 0, 2).astype(fp8)
    )
    v8 = np.repeat(
        (np.asarray(v_w, np.float32) * 64.0)
        .reshape(n_dt, 128).T.reshape(128, n_dt, 1).astype(fp8),
        2, axis=2,
    )
    ab_t = np.ascontiguousarray(
        np.asarray(a_b, np.float32).reshape(n_dt, 128).T
    )
    return (
        np.ascontiguousarray(w8),
        np.ascontiguousarray(wd8),
        np.ascontiguousarray(v8),
        ab_t,
    )


def kernel(hidden_states, encoder_outputs, encoder_masks, a_w, a_b, v_w):
    import ml_dtypes
    from concourse.bass_utils import run_bass_kernel_spmd

    if "nc" not in _CACHE:
        _CACHE["nc"] = build_bass_kernel()
    nc = _CACHE["nc"]

    bf16 = ml_dtypes.bfloat16
    fp8 = ml_dtypes.float8_e4m3
    hidden_states = np.asarray(hidden_states, dtype=np.float32)
    enc_f32 = np.asarray(encoder_outputs, dtype=np.float32)
    enc_bf = enc_f32.astype(bf16)
    enc8u = enc_f32.astype(fp8).view(np.uint16)
    encoder_masks = np.asarray(encoder_masks, dtype=np.int32)
    w8, wd8, v8, ab_t = _prep_weights(a_w, a_b, v_w)
    ident = np.eye(128, dtype=bf16)
    n_dt = DEC // 128

    in_maps = []
    for c in range(N_CORES):
        sl = slice(c * BC, (c + 1) * BC)
        hsT8 = np.ascontiguousarray(
            hidden_states[sl].T.reshape(n_dt, 128, BC).transpose(1, 0, 2)
        ).astype(fp8)
        m = {
            "enc_bf": np.ascontiguousarray(enc_bf[sl]),
            "enc8u": np.ascontiguousarray(enc8u[sl]),
            "encoder_masks": np.ascontiguousarray(encoder_masks[sl]),
            "w8": w8,
            "wd8": wd8,
            "hsT8": np.ascontiguousarray(hsT8),
            "ab_t": ab_t,
            "v8": v8,
            "ident": ident,
        }
        in_maps.append(m)

    global _LAST_IN_MAPS
    _LAST_IN_MAPS = in_maps
    res = run_bass_kernel_spmd(nc, in_maps, core_ids=list(range(N_CORES)))
    out = np.concatenate([r["out"] for r in res.results], axis=0)
    return out.astype(np.float32)


_LAST_IN_MAPS = None


# revision 14
# speedup vs baseline: 2.3884x; 1.2096x over previous
"""Bahdanau-style attention kernel for Trainium2 (8 NeuronCores, SPMD).

Math (per batch row b):
    h_proj = hidden @ a_w[:DEC]                       (DEC,)
    e_proj[s, :] = enc[s, :] @ a_w[DEC:]              (S, DEC)
    energy = tanh(e_proj + h_proj + a_b)              (S, DEC)
    scores = energy @ v_w                             (S,)
    scores = where(mask == 0, -1e10, scores)
    attn = softmax(scores)                            (S,)
    out = attn @ enc                                  (ENC,)

Sharding: data-parallel over batch (32 rows -> 4 rows on each of 8 cores);
weights replicated (pre-quantized to fp8*64 on host).

Per-core strategy:
  - The weighted sum runs in bf16 from natural-layout [tok, e] chunks
    (host-cast enc); softmax-averaging keeps per-element quantization
    error in the output, so fp8 enc there would blow the 2e-2 gate.
  - e_proj runs in fp8 with MatmulPerfMode.DoubleRow (2 k-tiles per
    instruction at 0.5 cycles/row). The transposed fp8 operand comes from
    the xbar DMA transpose moving fp8 PAIRS as uint16 lanes straight from
    DRAM: out[p, g, q](u16) = enc-pair(e=2(128g+p)(+0/1), tok q). The pair
    interleave is absorbed by the DoubleRow k-pair dimension with a
    host-permuted weight layout w8[p, g, i, d] = 64*a_w[DEC+2(128g+p)+i, d],
    so no on-chip bf16->fp8 cast and no bf16 staging is needed.
  - e_proj PSUM is [128d, 2x512tok] (a chunk pair, 2 banks) so one tanh
    activation covers 1024 tokens per d-tile, amortizing the ~185ns
    ScalarE access overhead; bias (h_proj + a_b) is per-partition.
  - scores = v . tanh as fp8 DoubleRow over d-tile pairs (v padded to
    M=2 / k-stride 512 for the dual-fp8 ldweights ISA restriction); exp
    is fused into the PSUM evacuation (scale=1/64 undoes the *64 weight
    scaling). Softmax tail on DVE.
  - The weighted sum accumulates chunk-major into one PSUM bank
    (e-halves at partitions 0/32) so nat buffers free chunk-by-chunk,
    and each iteration emits weighted(b-1) before eproj(b) so next-row
    DMA overlaps this row's PE work.
"""

import numpy as np
from contextlib import ExitStack

B, S, ENC, DEC = 32, 2048, 1024, 1024
N_CORES = 8
BC = B // N_CORES  # batch rows per core
CH = 512           # tokens per chunk


def build_bass_kernel(bc=BC, s=S, e_dim=ENC, d_dim=DEC, debug=False):
    import concourse.bass as bass
    import concourse.tile as tile
    from concourse import bacc, mybir

    f32 = mybir.dt.float32
    bf16 = mybir.dt.bfloat16
    fp8 = mybir.dt.float8e4
    u16 = mybir.dt.uint16
    i32 = mybir.dt.int32
    Tanh = mybir.ActivationFunctionType.Tanh
    Exp = mybir.ActivationFunctionType.Exp
    DR = mybir.MatmulPerfMode.DoubleRow

    assert s % (2 * CH) == 0 and e_dim % 256 == 0 and d_dim % 256 == 0
    n_chunks = s // CH             # 512-token chunks per batch row
    n_st = CH // 128               # s-tiles per chunk
    n_g = e_dim // 256             # e pair-groups (256 e-rows per group)
    n_dt = d_dim // 128            # d (output) tiles for e_proj
    n_ec = e_dim // 512            # 512-wide e chunks for the weighted sum
    n_ct = s // 128                # s-tiles per row

    nc = bacc.Bacc("TRN2", target_bir_lowering=False, debug=debug)

    enc_h = nc.dram_tensor("enc_bf", [bc, s, e_dim], bf16, kind="ExternalInput")
    enc8_h = nc.dram_tensor("enc8u", [bc, s, e_dim // 2], u16, kind="ExternalInput")
    mskT_h = nc.dram_tensor("maskT", [bc, 128, s // 128], bf16, kind="ExternalInput")
    w8_h = nc.dram_tensor("w8", [128, n_g, 2, d_dim], fp8, kind="ExternalInput")
    wd8_h = nc.dram_tensor("wd8", [128, n_dt, d_dim], fp8, kind="ExternalInput")
    hsT8_h = nc.dram_tensor("hsT8", [128, n_dt, bc], fp8, kind="ExternalInput")
    ab_h = nc.dram_tensor("ab_t", [128, n_dt], f32, kind="ExternalInput")
    v8_h = nc.dram_tensor("v8", [128, n_dt, 2], fp8, kind="ExternalInput")
    id_h = nc.dram_tensor("ident", [128, 128], bf16, kind="ExternalInput")
    out_h = nc.dram_tensor("out", [bc, e_dim], f32, kind="ExternalOutput")

    with tile.TileContext(nc) as tc, ExitStack() as ctx:
        consts = ctx.enter_context(tc.tile_pool(name="consts", bufs=1))
        nat_pool = ctx.enter_context(tc.tile_pool(name="nat", bufs=2 * n_chunks))
        eT_pool = ctx.enter_context(tc.tile_pool(name="eT", bufs=2 * n_chunks + 1))
        th_pool = ctx.enter_context(tc.tile_pool(name="th", bufs=2))
        sm_pool = ctx.enter_context(tc.tile_pool(name="softmax", bufs=2))
        small_pool = ctx.enter_context(tc.tile_pool(name="small", bufs=4))
        outsb_pool = ctx.enter_context(tc.tile_pool(name="outsb", bufs=1))
        pe_psum = ctx.enter_context(tc.tile_pool(name="pe_psum", bufs=2, space="PSUM"))
        sc_psum = ctx.enter_context(tc.tile_pool(name="sc_psum", bufs=1, space="PSUM"))
        w_psum = ctx.enter_context(tc.tile_pool(name="w_psum", bufs=1, space="PSUM"))
        misc_psum = ctx.enter_context(tc.tile_pool(name="misc_psum", bufs=2, space="PSUM"))

        # ---------------- consts ----------------
        ident_sb = consts.tile([128, 128], bf16)
        nc.sync.dma_start(out=ident_sb, in_=id_h[:, :])
        ones_bf = ident_sb[0:1, 0:1]
        ones_f = consts.tile([128, 1], f32)
        nc.vector.memset(ones_f, 1.0)

        w8_sb = consts.tile([128, n_g, 2, d_dim], fp8)
        # dual-fp8 ldweights needs a wide stride between the k-pair weight
        # blocks (walrus s3_lw_dual_fp8_restrictions rejects stride 2/4;
        # 512 verified on HW) -> stage v into a padded tile
        v8_sb = consts.tile([128, n_dt, 512], fp8)
        ab_sb = consts.tile([128, n_dt], f32)
        hsT8_sb = consts.tile([128, n_dt, bc], fp8)
        wd8_sb = consts.tile([128, n_dt, d_dim], fp8)

        def emit_consts(step):
            # spread const loads between row-0 chunk work so the first
            # e_proj matmuls aren't starved behind them on the DMA device
            if step == 0:
                nc.sync.dma_start(out=w8_sb, in_=w8_h[:, :, :, :])
            elif step == 1:
                nc.sync.dma_start(out=wd8_sb, in_=wd8_h[:, :, :])
                nc.sync.dma_start(out=hsT8_sb, in_=hsT8_h[:, :, :])
                nc.sync.dma_start(out=ab_sb, in_=ab_h[:, :])
            elif step == 2:
                nc.sync.dma_start(out=v8_sb[:, :, 0:2], in_=v8_h[:, :, :])

        hb_sb = consts.tile([128, n_dt, bc], f32)

        state = {}

        def emit_xbar_chunk(b, c):
            # transpose fp8 pairs (as u16 lanes) straight from DRAM:
            # out[p, j, g, q] = enc8u[b, CH*c + 128*j + q, 128*g + p]
            eT = eT_pool.tile([128, n_st, n_g, 128], u16, tag="eT")
            for j in range(n_st):
                nc.sync.dma_start(
                    out=eT[:, j, :, :],
                    in_=enc8_h[b, CH * c + 128 * j : CH * c + 128 * (j + 1), :],
                    transpose=True,
                )
            state[(b, c)] = dict(eT=eT)

        def emit_load_chunk(b, c):
            nat = nat_pool.tile([128, n_st, e_dim], bf16, tag="nat")
            nc.sync.dma_start(
                out=nat,
                in_=enc_h[b, CH * c : CH * (c + 1), :].rearrange(
                    "(j p) e -> p j e", p=128
                ),
            )
            state[(b, c)]["nat"] = nat

        def emit_hproj():
            hp = misc_psum.tile([128, n_dt, bc], f32, tag="misc")
            for i in range(n_dt):
                for u in range(n_dt // 2):
                    nc.tensor.matmul(
                        hp[:, i, :],
                        lhsT=wd8_sb[:, 2 * u : 2 * u + 2, 128 * i : 128 * (i + 1)],
                        rhs=hsT8_sb[:, 2 * u : 2 * u + 2, :],
                        start=(u == 0),
                        stop=(u == n_dt // 2 - 1),
                        perf_mode=DR,
                    )
            for i in range(n_dt):
                nc.vector.tensor_scalar(
                    hb_sb[:, i, :], hp[:, i, :], 1.0 / 64,
                    ab_sb[:, i : i + 1],
                    op0=mybir.AluOpType.mult, op1=mybir.AluOpType.add,
                )

        def emit_eproj_pair(b, cp, mid_hook=None):
            eT = []
            for c in (2 * cp, 2 * cp + 1):
                # [p, j, g, q](u16) -> fp8 [p, j, g, (q two)]; per (j, g) the
                # DoubleRow rhs is [p, two, q]
                eT.append(state[(b, c)]["eT"][:, :, :, :].bitcast(fp8))
            scores = state[b]["scores"]
            th = th_pool.tile([128, n_dt, 2 * CH], fp8, tag="th")
            for i in range(n_dt):
                ps = pe_psum.tile([128, 2, CH], f32, tag="pe")
                for h in range(2):
                    for j in range(n_st):
                        for g in range(n_g):
                            rhs = eT[h][:, j, g, :].rearrange(
                                "p (q two) -> p two q", two=2
                            )
                            nc.tensor.matmul(
                                ps[:, h, 128 * j : 128 * (j + 1)],
                                lhsT=w8_sb[:, g, :, 128 * i : 128 * (i + 1)],
                                rhs=rhs,
                                start=(g == 0),
                                stop=(g == n_g - 1),
                                perf_mode=DR,
                            )
                if mid_hook is not None:
                    # h_proj needs to land before the first tanh reads hb
                    # (program-order RAW), but after d-tile 0's matmuls so
                    # the PE ramps on e_proj while w_dec arrives
                    mid_hook()
                    mid_hook = None
                nc.scalar.activation(
                    th[:, i, :], ps, Tanh, bias=hb_sb[:, i, b : b + 1], scale=1.0 / 64
                )
            for h in range(2):
                sc = sc_psum.tile([2, CH], f32, tag="sc")
                for m in range(n_dt // 2):
                    nc.tensor.matmul(
                        sc,
                        lhsT=v8_sb[:, 2 * m : 2 * m + 2, 0:2],
                        rhs=th[:, 2 * m : 2 * m + 2, CH * h : CH * (h + 1)],
                        start=(m == 0),
                        stop=(m == n_dt // 2 - 1),
                        perf_mode=DR,
                    )
                pos = CH * (2 * cp + h)
                nc.scalar.activation(
                    scores[:, pos : pos + CH], sc[0:1, :], Exp, bias=0.0, scale=1.0 / 64
                )

        def emit_row_prep(b):
            maskT = sm_pool.tile([128, n_ct], bf16, tag="maskT")
            nc.sync.dma_start(out=maskT, in_=mskT_h[b, :, :])
            scores = sm_pool.tile([1, s], bf16, tag="scores")
            state[b] = dict(maskT=maskT, scores=scores)

        def emit_weighted_part(b, cp):
            # transpose this pair's exp(scores) into columns, apply the mask
            # during psum evacuation, then accumulate the (unnormalized)
            # weighted sum for the pair's two chunks. The 1/sum normalizer is
            # folded into the final output evacuation, so none of this waits
            # on a full-row softmax.
            scores = state[b]["scores"]
            maskT = state[b]["maskT"]
            half = n_ct // 2
            j0 = cp * half
            psum_at = misc_psum.tile([128, half], f32, tag="misc")
            for j in range(half):
                nc.tensor.matmul(
                    psum_at[:, j : j + 1],
                    lhsT=scores[:, 128 * (j0 + j) : 128 * (j0 + j + 1)],
                    rhs=ones_bf,
                    start=True,
                    stop=True,
                )
            if cp == 0:
                attnT_new = small_pool.tile([128, n_ct], bf16, tag="attnT")
                pw_new = w_psum.tile([128, 512], f32, tag="w")
                state[b]["attnT"] = attnT_new
                state[b]["pw"] = pw_new
            attnT = state[b]["attnT"]
            pw = state[b]["pw"]
            nc.vector.tensor_mul(
                attnT[:, j0 : j0 + half], psum_at, maskT[:, j0 : j0 + half]
            )
            for c in range(2 * cp, 2 * cp + 2):
                for ec in range(n_ec):
                    for jj in range(n_st):
                        nc.tensor.matmul(
                            pw[32 * ec : 32 * ec + 1, :],
                            lhsT=attnT[:, c * n_st + jj : c * n_st + jj + 1],
                            rhs=state[(b, c)]["nat"][:, jj, 512 * ec : 512 * (ec + 1)],
                            start=(c == 0 and jj == 0),
                            stop=(c == n_chunks - 1 and jj == n_st - 1),
                        )

        def emit_weighted_finish(b):
            attnT = state[b]["attnT"]
            pw = state[b]["pw"]
            partials = small_pool.tile([128, 1], f32, tag="part")
            nc.vector.reduce_sum(out=partials, in_=attnT, axis=mybir.AxisListType.X)
            psum_s = misc_psum.tile([1, 1], f32, tag="misc")
            nc.tensor.matmul(
                psum_s, lhsT=partials, rhs=ones_f, start=True, stop=True
            )
            rsum = small_pool.tile([1, 1], f32, tag="rsum")
            nc.vector.reciprocal(rsum, psum_s)
            out_sb = outsb_pool.tile([1, e_dim], f32, tag="outsb")
            for ec in range(n_ec):
                nc.vector.tensor_scalar_mul(
                    out_sb[:, 512 * ec : 512 * (ec + 1)],
                    pw[32 * ec : 32 * ec + 1, :],
                    rsum[0:1, 0:1],
                )
            nc.sync.dma_start(out=out_h[b : b + 1, :], in_=out_sb)

        # ---------------- schedule ----------------
        emit_consts(0)
        emit_xbar_chunk(0, 0)
        emit_xbar_chunk(0, 1)
        emit_consts(1)
        emit_xbar_chunk(0, 2)
        emit_xbar_chunk(0, 3)
        emit_consts(2)
        emit_row_prep(0)
        for c in range(n_chunks):
            emit_load_chunk(0, c)
        for b in range(bc):
            if b + 1 < bc:
                emit_row_prep(b + 1)
                for c in range(n_chunks):
                    emit_xbar_chunk(b + 1, c)
                for c in range(n_chunks):
                    emit_load_chunk(b + 1, c)
            # one-pair stagger: the weighted accumulation for pair cp is
            # emitted after eproj of the NEXT pair, so the PE never waits on
            # the Activation engine's exp of the pair it just produced
            emit_eproj_pair(b, 0, mid_hook=emit_hproj if b == 0 else None)
            if b >= 1:
                emit_weighted_part(b - 1, 1)
                emit_weighted_finish(b - 1)
            emit_eproj_pair(b, 1)
            emit_weighted_part(b, 0)
        emit_weighted_part(bc - 1, 1)
        emit_weighted_finish(bc - 1)

    nc.compile()
    return nc


_CACHE = {}


def _prep_weights(a_w, a_b, v_w, e_dim=ENC, d_dim=DEC):
    import ml_dtypes

    fp8 = ml_dtypes.float8_e4m3
    n_g, n_dt = e_dim // 256, d_dim // 128
    # w8[p, g, i, d] = 64 * a_w[DEC + 2*(128*g + p) + i, d]
    w8 = (
        (np.asarray(a_w[d_dim:], np.float32) * 64.0)
        .reshape(n_g, 128, 2, d_dim).transpose(1, 0, 2, 3).astype(fp8)
    )
    wd8 = (
        (np.asarray(a_w[:d_dim], np.float32) * 64.0)
        .reshape(n_dt, 128, d_dim).transpose(1, 0, 2).astype(fp8)
    )
    v8 = np.repeat(
        (np.asarray(v_w, np.float32) * 64.0)
        .reshape(n_dt, 128).T.reshape(128, n_dt, 1).astype(fp8),
        2, axis=2,
    )
    ab_t = np.ascontiguousarray(
        np.asarray(a_b, np.float32).reshape(n_dt, 128).T
    )
    return (
        np.ascontiguousarray(w8),
        np.ascontiguousarray(wd8),
        np.ascontiguousarray(v8),
        ab_t,
    )


def kernel(hidden_states, encoder_outputs, encoder_masks, a_w, a_b, v_w):
    import ml_dtypes
    from concourse.bass_utils import run_bass_kernel_spmd

    if "nc" not in _CACHE:
        _CACHE["nc"] = build_bass_kernel()
    nc = _CACHE["nc"]

    bf16 = ml_dtypes.bfloat16
    fp8 = ml_dtypes.float8_e4m3
    hidden_states = np.asarray(hidden_states, dtype=np.float32)
    enc_f32 = np.asarray(encoder_outputs, dtype=np.float32)
    enc_bf = enc_f32.astype(bf16)
    enc8u = enc_f32.astype(fp8).view(np.uint16)
    encoder_masks = np.asarray(encoder_masks, dtype=np.int32)
    w8, wd8, v8, ab_t = _prep_weights(a_w, a_b, v_w)
    ident = np.eye(128, dtype=bf16)
    n_dt = DEC // 128

    in_maps = []
    for c in range(N_CORES):
        sl = slice(c * BC, (c + 1) * BC)
        hsT8 = np.ascontiguousarray(
            hidden_states[sl].T.reshape(n_dt, 128, BC).transpose(1, 0, 2)
        ).astype(fp8)
        maskT = np.ascontiguousarray(
            (encoder_masks[sl] != 0)
            .reshape(BC, S // 128, 128).transpose(0, 2, 1)
        ).astype(bf16)
        m = {
            "enc_bf": np.ascontiguousarray(enc_bf[sl]),
            "enc8u": np.ascontiguousarray(enc8u[sl]),
            "maskT": maskT,
            "w8": w8,
            "wd8": wd8,
            "hsT8": np.ascontiguousarray(hsT8),
            "ab_t": ab_t,
            "v8": v8,
            "ident": ident,
        }
        in_maps.append(m)

    global _LAST_IN_MAPS
    _LAST_IN_MAPS = in_maps
    res = run_bass_kernel_spmd(nc, in_maps, core_ids=list(range(N_CORES)))
    out = np.concatenate([r["out"] for r in res.results], axis=0)
    return out.astype(np.float32)


_LAST_IN_MAPS = None


# revision 18
# speedup vs baseline: 2.6614x; 1.1143x over previous
"""Bahdanau-style attention kernel for Trainium2 (8 NeuronCores, SPMD).

Math (per batch row b):
    h_proj = hidden @ a_w[:DEC]                       (DEC,)
    e_proj[s, :] = enc[s, :] @ a_w[DEC:]              (S, DEC)
    energy = tanh(e_proj + h_proj + a_b)              (S, DEC)
    scores = energy @ v_w                             (S,)
    scores = where(mask == 0, -1e10, scores)
    attn = softmax(scores)                            (S,)
    out = attn @ enc                                  (ENC,)

Sharding: data-parallel over batch (32 rows -> 4 rows on each of 8 cores);
weights replicated (pre-quantized to fp8*64 on host).

Per-core strategy:
  - The weighted sum runs in bf16 from natural-layout [tok, e] chunks
    (host-cast enc); softmax-averaging keeps per-element quantization
    error in the output, so fp8 enc there would blow the 2e-2 gate.
  - e_proj runs in fp8 with MatmulPerfMode.DoubleRow (2 k-tiles per
    instruction at 0.5 cycles/row). The transposed fp8 operand comes from
    the xbar DMA transpose moving fp8 PAIRS as uint16 lanes straight from
    DRAM: out[p, g, q](u16) = enc-pair(e=2(128g+p)(+0/1), tok q). The pair
    interleave is absorbed by the DoubleRow k-pair dimension with a
    host-permuted weight layout w8[p, g, i, d] = 64*a_w[DEC+2(128g+p)+i, d],
    so no on-chip bf16->fp8 cast and no bf16 staging is needed.
  - e_proj PSUM is [128d, 2x512tok] (a chunk pair, 2 banks) so one tanh
    activation covers 1024 tokens per d-tile, amortizing the ~185ns
    ScalarE access overhead; bias (h_proj + a_b) is per-partition.
  - scores = v . tanh as fp8 DoubleRow over d-tile pairs (v padded to
    M=2 / k-stride 512 for the dual-fp8 ldweights ISA restriction); exp
    is fused into the PSUM evacuation (scale=1/64 undoes the *64 weight
    scaling). Softmax tail on DVE.
  - The weighted sum accumulates chunk-major into one PSUM bank
    (e-halves at partitions 0/32) so nat buffers free chunk-by-chunk,
    and each iteration emits weighted(b-1) before eproj(b) so next-row
    DMA overlaps this row's PE work.
"""

import numpy as np
from contextlib import ExitStack

B, S, ENC, DEC = 32, 2048, 1024, 1024
N_CORES = 8
BC = B // N_CORES  # batch rows per core
CH = 512           # tokens per chunk


def build_bass_kernel(bc=BC, s=S, e_dim=ENC, d_dim=DEC, debug=False):
    import concourse.bass as bass
    import concourse.tile as tile
    from concourse import bacc, mybir

    f32 = mybir.dt.float32
    bf16 = mybir.dt.bfloat16
    fp8 = mybir.dt.float8e4
    u16 = mybir.dt.uint16
    i32 = mybir.dt.int32
    Tanh = mybir.ActivationFunctionType.Tanh
    Exp = mybir.ActivationFunctionType.Exp
    DR = mybir.MatmulPerfMode.DoubleRow

    assert s % (2 * CH) == 0 and e_dim % 256 == 0 and d_dim % 256 == 0
    n_chunks = s // CH             # 512-token chunks per batch row
    n_st = CH // 128               # s-tiles per chunk
    n_g = e_dim // 256             # e pair-groups (256 e-rows per group)
    n_dt = d_dim // 128            # d (output) tiles for e_proj
    n_ec = e_dim // 512            # 512-wide e chunks for the weighted sum
    n_ct = s // 128                # s-tiles per row

    nc = bacc.Bacc("TRN2", target_bir_lowering=False, debug=debug)

    ench_h = nc.dram_tensor("ench", [bc, s, e_dim], fp8, kind="ExternalInput")
    encl_h = nc.dram_tensor("encl", [bc, s, e_dim], fp8, kind="ExternalInput")
    mskT_h = nc.dram_tensor("maskT", [bc, 128, s // 128], bf16, kind="ExternalInput")
    w8_h = nc.dram_tensor("w8", [128, n_g, 2, d_dim], fp8, kind="ExternalInput")
    wd8_h = nc.dram_tensor("wd8", [128, n_dt, d_dim], fp8, kind="ExternalInput")
    hsT8_h = nc.dram_tensor("hsT8", [128, n_dt, bc], fp8, kind="ExternalInput")
    ab_h = nc.dram_tensor("ab_t", [128, n_dt], f32, kind="ExternalInput")
    v8_h = nc.dram_tensor("v8", [128, n_dt, 2], fp8, kind="ExternalInput")
    id_h = nc.dram_tensor("ident", [128, 128], bf16, kind="ExternalInput")
    out_h = nc.dram_tensor("out", [bc, e_dim], f32, kind="ExternalOutput")

    with tile.TileContext(nc) as tc, ExitStack() as ctx:
        consts = ctx.enter_context(tc.tile_pool(name="consts", bufs=1))
        nat_pool = ctx.enter_context(tc.tile_pool(name="nat", bufs=2 * n_chunks))
        eT_pool = ctx.enter_context(tc.tile_pool(name="eT", bufs=2 * n_chunks + 1))
        th_pool = ctx.enter_context(tc.tile_pool(name="th", bufs=2))
        sm_pool = ctx.enter_context(tc.tile_pool(name="softmax", bufs=2))
        small_pool = ctx.enter_context(tc.tile_pool(name="small", bufs=4))
        outsb_pool = ctx.enter_context(tc.tile_pool(name="outsb", bufs=1))
        pe_psum = ctx.enter_context(tc.tile_pool(name="pe_psum", bufs=2, space="PSUM"))
        sc_psum = ctx.enter_context(tc.tile_pool(name="sc_psum", bufs=2, space="PSUM"))
        w_psum = ctx.enter_context(tc.tile_pool(name="w_psum", bufs=1, space="PSUM"))

        # ---------------- consts ----------------
        ident_sb = consts.tile([128, 128], bf16)
        nc.sync.dma_start(out=ident_sb, in_=id_h[:, :])
        ones_bf = ident_sb[0:1, 0:1]
        ones_f = consts.tile([128, 1], f32)
        nc.vector.memset(ones_f, 1.0)

        w8_sb = consts.tile([128, n_g, 2, d_dim], fp8)
        # dual-fp8 ldweights needs a wide stride between the k-pair weight
        # blocks (walrus s3_lw_dual_fp8_restrictions rejects stride 2/4;
        # 512 verified on HW) -> stage v into a padded tile
        v8_sb = consts.tile([128, n_dt, 512], fp8)
        ab_sb = consts.tile([128, n_dt], f32)
        hsT8_sb = consts.tile([128, n_dt, bc], fp8)
        wd8_sb = consts.tile([128, n_dt, d_dim], fp8)

        def emit_consts(step):
            # spread const loads between row-0 chunk work so the first
            # e_proj matmuls aren't starved behind them on the DMA device
            if step == 0:
                nc.sync.dma_start(out=w8_sb, in_=w8_h[:, :, :, :])
            elif step == 1:
                nc.sync.dma_start(out=wd8_sb, in_=wd8_h[:, :, :])
                nc.sync.dma_start(out=hsT8_sb, in_=hsT8_h[:, :, :])
                nc.sync.dma_start(out=ab_sb, in_=ab_h[:, :])
            elif step == 2:
                nc.sync.dma_start(out=v8_sb[:, :, 0:2], in_=v8_h[:, :, :])

        hb_sb = consts.tile([128, n_dt, bc], f32)

        state = {}

        def emit_xbar_chunk(b, c):
            # transpose fp8 pairs (as u16 lanes) straight from DRAM, one
            # 512-row instruction per chunk:
            # out[p, g, q] = enc8u[b, CH*c + q, 128*g + p]
            eT = eT_pool.tile([128, n_g, CH], u16, tag="eT")
            nc.sync.dma_start(
                out=eT,
                in_=ench_h[b, CH * c : CH * (c + 1), :].bitcast(u16),
                transpose=True,
            )
            state[(b, c)] = dict(eT=eT)

        def emit_load_chunk(b, c):
            nath = nat_pool.tile([128, n_st, e_dim], fp8, tag="nath")
            nc.sync.dma_start(
                out=nath,
                in_=ench_h[b, CH * c : CH * (c + 1), :].rearrange(
                    "(j p) e -> p j e", p=128
                ),
            )
            natl = nat_pool.tile([128, n_st, e_dim], fp8, tag="natl")
            nc.sync.dma_start(
                out=natl,
                in_=encl_h[b, CH * c : CH * (c + 1), :].rearrange(
                    "(j p) e -> p j e", p=128
                ),
            )
            state[(b, c)]["nath"] = nath
            state[(b, c)]["natl"] = natl

        def emit_hproj():
            hp = sc_psum.tile([128, n_dt, bc], f32, tag="sc")
            for i in range(n_dt):
                for u in range(n_dt // 2):
                    nc.tensor.matmul(
                        hp[:, i, :],
                        lhsT=wd8_sb[:, 2 * u : 2 * u + 2, 128 * i : 128 * (i + 1)],
                        rhs=hsT8_sb[:, 2 * u : 2 * u + 2, :],
                        start=(u == 0),
                        stop=(u == n_dt // 2 - 1),
                        perf_mode=DR,
                    )
            for i in range(n_dt):
                nc.vector.tensor_scalar(
                    hb_sb[:, i, :], hp[:, i, :], 1.0 / 64,
                    ab_sb[:, i : i + 1],
                    op0=mybir.AluOpType.mult, op1=mybir.AluOpType.add,
                )

        def emit_eproj_pair(b, cp, mid_hook=None):
            eT = []
            for c in (2 * cp, 2 * cp + 1):
                # [p, g, q](u16) -> fp8 [p, g, (q two)]; per (j, g) the
                # DoubleRow rhs is [p, two, q]
                eT.append(state[(b, c)]["eT"][:, :, :].bitcast(fp8))
            th = th_pool.tile([128, n_dt, 2 * CH], fp8, tag="th")
            for i in range(n_dt):
                ps = pe_psum.tile([128, 2, CH], f32, tag="pe")
                for h in range(2):
                    for j in range(n_st):
                        for g in range(n_g):
                            rhs = eT[h][:, g, 256 * j : 256 * (j + 1)].rearrange(
                                "p (q two) -> p two q", two=2
                            )
                            nc.tensor.matmul(
                                ps[:, h, 128 * j : 128 * (j + 1)],
                                lhsT=w8_sb[:, g, :, 128 * i : 128 * (i + 1)],
                                rhs=rhs,
                                start=(g == 0),
                                stop=(g == n_g - 1),
                                perf_mode=DR,
                            )
                if mid_hook is not None:
                    # h_proj needs to land before the first tanh reads hb
                    # (program-order RAW), but after d-tile 0's matmuls so
                    # the PE ramps on e_proj while w_dec arrives
                    mid_hook()
                    mid_hook = None
                nc.scalar.activation(
                    th[:, i, :], ps, Tanh, bias=hb_sb[:, i, b : b + 1], scale=1.0 / 64
                )
            state[(b, cp, "th")] = th

        def emit_scores(b, cp):
            th = state[(b, cp, "th")]
            scores = state[b]["scores"]
            for h in range(2):
                sc = sc_psum.tile([2, CH], f32, tag="sc")
                for m in range(n_dt // 2):
                    nc.tensor.matmul(
                        sc,
                        lhsT=v8_sb[:, 2 * m : 2 * m + 2, 0:2],
                        rhs=th[:, 2 * m : 2 * m + 2, CH * h : CH * (h + 1)],
                        start=(m == 0),
                        stop=(m == n_dt // 2 - 1),
                        perf_mode=DR,
                    )
                pos = CH * (2 * cp + h)
                nc.scalar.activation(
                    scores[:, pos : pos + CH], sc[0:1, :], Exp, bias=0.0, scale=1.0 / 64
                )

        def emit_row_prep(b):
            maskT = sm_pool.tile([128, n_ct], bf16, tag="maskT")
            nc.sync.dma_start(out=maskT, in_=mskT_h[b, :, :])
            scores = sm_pool.tile([1, s], bf16, tag="scores")
            state[b] = dict(maskT=maskT, scores=scores)

        def emit_weighted_part(b, cp):
            # transpose this pair's exp(scores) into columns, apply the mask
            # during psum evacuation, then accumulate the (unnormalized)
            # weighted sum for the pair's two chunks. The 1/sum normalizer is
            # folded into the final output evacuation, so none of this waits
            # on a full-row softmax.
            scores = state[b]["scores"]
            maskT = state[b]["maskT"]
            half = n_ct // 2
            j0 = cp * half
            psum_at = sc_psum.tile([128, half], f32, tag="sc")
            for j in range(half):
                nc.tensor.matmul(
                    psum_at[:, j : j + 1],
                    lhsT=scores[:, 128 * (j0 + j) : 128 * (j0 + j + 1)],
                    rhs=ones_bf,
                    start=True,
                    stop=True,
                )
            if cp == 0:
                attnT_new = small_pool.tile([128, n_ct], bf16, tag="attnT")
                ah_new = small_pool.tile([128, 2, 512], fp8, tag="ah")
                al_new = small_pool.tile([128, 2, 512], fp8, tag="al")
                pw_new = w_psum.tile([1, 2, 512], f32, tag="w")
                state[b]["attnT"] = attnT_new
                state[b]["ah"] = ah_new
                state[b]["al"] = al_new
                state[b]["pw"] = pw_new
            attnT = state[b]["attnT"]
            ah, al = state[b]["ah"], state[b]["al"]
            pw = state[b]["pw"]
            nc.vector.tensor_mul(
                attnT[:, j0 : j0 + half], psum_at, maskT[:, j0 : j0 + half]
            )
            # split attn into fp8 + fp8 residual, packed by s-tile-pair parity
            # with a 512 k-stride (dual-fp8 ldweights wants wide strides);
            # ah[p, i, u] = attn(tok=(2u+i)*128+p)
            u0 = half // 2 * cp
            nu = half // 2
            asrc = attnT[:, j0 : j0 + half].rearrange("p (u two) -> p two u", two=2)
            nc.vector.tensor_copy(out=ah[:, :, u0 : u0 + nu], in_=asrc)
            nc.vector.tensor_tensor(
                out=al[:, :, u0 : u0 + nu], in0=asrc, in1=ah[:, :, u0 : u0 + nu],
                op=mybir.AluOpType.subtract,
            )
            # hi*hi + hi*lo + lo*hi accumulate into one psum group
            # (residuals are unscaled fp8, so no rescale is needed)
            first_u, last_u = n_ct // 4 * cp, n_ct // 4 * (cp + 1) - 1
            for ec in range(n_ec):
                for u in range(first_u, last_u + 1):
                    c, jj = divmod(2 * u, n_st)
                    nath = state[(b, c)]["nath"][:, jj : jj + 2, 512 * ec : 512 * (ec + 1)]
                    natl = state[(b, c)]["natl"][:, jj : jj + 2, 512 * ec : 512 * (ec + 1)]
                    for src_a, src_e, is_first, is_last in (
                        (ah, nath, cp == 0 and u == first_u, False),
                        (ah, natl, False, False),
                        (al, nath, False, cp == n_chunks // 2 - 1 and u == last_u),
                    ):
                        nc.tensor.matmul(
                            pw[:, ec, :],
                            lhsT=src_a[:, :, u : u + 1],
                            rhs=src_e,
                            start=is_first,
                            stop=is_last,
                            perf_mode=DR,
                        )

        def emit_weighted_finish(b):
            attnT = state[b]["attnT"]
            pw = state[b]["pw"]
            partials = small_pool.tile([128, 1], f32, tag="part")
            nc.vector.reduce_sum(out=partials, in_=attnT, axis=mybir.AxisListType.X)
            psum_s = sc_psum.tile([1, 1], f32, tag="sc")
            nc.tensor.matmul(
                psum_s, lhsT=partials, rhs=ones_f, start=True, stop=True
            )
            rsum = small_pool.tile([1, 1], f32, tag="rsum")
            nc.vector.reciprocal(rsum, psum_s)
            out_sb = outsb_pool.tile([1, e_dim], f32, tag="outsb")
            for ec in range(n_ec):
                nc.vector.tensor_scalar_mul(
                    out_sb[:, 512 * ec : 512 * (ec + 1)],
                    pw[:, ec, :],
                    rsum[0:1, 0:1],
                )
            nc.sync.dma_start(out=out_h[b : b + 1, :], in_=out_sb)

        # ---------------- schedule ----------------
        emit_consts(0)
        emit_xbar_chunk(0, 0)
        emit_xbar_chunk(0, 1)
        emit_consts(1)
        emit_row_prep(0)
        emit_xbar_chunk(0, 2)
        emit_xbar_chunk(0, 3)
        emit_consts(2)
        for c in range(n_chunks):
            emit_load_chunk(0, c)
        # one-pair stagger across the whole pipeline: while the PE runs
        # eproj of pair P, it then retires scores/exp/transpose/weighted of
        # pair P-1, whose Activation-side work completed during eproj(P) --
        # the PE never waits on ScalarE.
        pairs = [(b, cp) for b in range(bc) for cp in range(n_chunks // 2)]
        for idx, (b, cp) in enumerate(pairs):
            if cp == 0 and b + 1 < bc:
                emit_row_prep(b + 1)
                for c in range(n_chunks):
                    emit_xbar_chunk(b + 1, c)
                for c in range(n_chunks):
                    emit_load_chunk(b + 1, c)
            emit_eproj_pair(b, cp, mid_hook=emit_hproj if idx == 0 else None)
            if idx >= 1:
                pb, pcp = pairs[idx - 1]
                emit_scores(pb, pcp)
                emit_weighted_part(pb, pcp)
                if pcp == n_chunks // 2 - 1:
                    emit_weighted_finish(pb)
        pb, pcp = pairs[-1]
        emit_scores(pb, pcp)
        emit_weighted_part(pb, pcp)
        emit_weighted_finish(pb)

    nc.compile()
    return nc


_CACHE = {}


def _prep_weights(a_w, a_b, v_w, e_dim=ENC, d_dim=DEC):
    import ml_dtypes

    fp8 = ml_dtypes.float8_e4m3
    n_g, n_dt = e_dim // 256, d_dim // 128
    # w8[p, g, i, d] = 64 * a_w[DEC + 2*(128*g + p) + i, d]
    w8 = (
        (np.asarray(a_w[d_dim:], np.float32) * 64.0)
        .reshape(n_g, 128, 2, d_dim).transpose(1, 0, 2, 3).astype(fp8)
    )
    wd8 = (
        (np.asarray(a_w[:d_dim], np.float32) * 64.0)
        .reshape(n_dt, 128, d_dim).transpose(1, 0, 2).astype(fp8)
    )
    v8 = np.repeat(
        (np.asarray(v_w, np.float32) * 64.0)
        .reshape(n_dt, 128).T.reshape(128, n_dt, 1).astype(fp8),
        2, axis=2,
    )
    ab_t = np.ascontiguousarray(
        np.asarray(a_b, np.float32).reshape(n_dt, 128).T
    )
    return (
        np.ascontiguousarray(w8),
        np.ascontiguousarray(wd8),
        np.ascontiguousarray(v8),
        ab_t,
    )


def kernel(hidden_states, encoder_outputs, encoder_masks, a_w, a_b, v_w):
    import ml_dtypes
    from concourse.bass_utils import run_bass_kernel_spmd

    if "nc" not in _CACHE:
        _CACHE["nc"] = build_bass_kernel()
    nc = _CACHE["nc"]

    bf16 = ml_dtypes.bfloat16
    fp8 = ml_dtypes.float8_e4m3
    hidden_states = np.asarray(hidden_states, dtype=np.float32)
    enc_f32 = np.asarray(encoder_outputs, dtype=np.float32)
    ench = enc_f32.astype(fp8)
    encl = (enc_f32 - ench.astype(np.float32)).astype(fp8)
    encoder_masks = np.asarray(encoder_masks, dtype=np.int32)
    w8, wd8, v8, ab_t = _prep_weights(a_w, a_b, v_w)
    ident = np.eye(128, dtype=bf16)
    n_dt = DEC // 128

    in_maps = []
    for c in range(N_CORES):
        sl = slice(c * BC, (c + 1) * BC)
        hsT8 = np.ascontiguousarray(
            hidden_states[sl].T.reshape(n_dt, 128, BC).transpose(1, 0, 2)
        ).astype(fp8)
        maskT = np.ascontiguousarray(
            (encoder_masks[sl] != 0)
            .reshape(BC, S // 128, 128).transpose(0, 2, 1)
        ).astype(bf16)
        m = {
            "ench": np.ascontiguousarray(ench[sl]),
            "encl": np.ascontiguousarray(encl[sl]),
            "maskT": maskT,
            "w8": w8,
            "wd8": wd8,
            "hsT8": np.ascontiguousarray(hsT8),
            "ab_t": ab_t,
            "v8": v8,
            "ident": ident,
        }
        in_maps.append(m)

    global _LAST_IN_MAPS
    _LAST_IN_MAPS = in_maps
    res = run_bass_kernel_spmd(nc, in_maps, core_ids=list(range(N_CORES)))
    out = np.concatenate([r["out"] for r in res.results], axis=0)
    return out.astype(np.float32)


_LAST_IN_MAPS = None


# revision 32
# speedup vs baseline: 2.7190x; 1.0216x over previous
"""Bahdanau-style attention kernel for Trainium2 (8 NeuronCores, SPMD).

Math (per batch row b):
    h_proj = hidden @ a_w[:DEC]                       (DEC,)
    e_proj[s, :] = enc[s, :] @ a_w[DEC:]              (S, DEC)
    energy = tanh(e_proj + h_proj + a_b)              (S, DEC)
    scores = energy @ v_w                             (S,)
    scores = where(mask == 0, -1e10, scores)
    attn = softmax(scores)                            (S,)
    out = attn @ enc                                  (ENC,)

Sharding: data-parallel over batch (32 rows -> 4 rows on each of 8 cores);
weights replicated (pre-quantized to fp8*64 on host).

Per-core strategy:
  - The weighted sum runs in bf16 from natural-layout [tok, e] chunks
    (host-cast enc); softmax-averaging keeps per-element quantization
    error in the output, so fp8 enc there would blow the 2e-2 gate.
  - e_proj runs in fp8 with MatmulPerfMode.DoubleRow (2 k-tiles per
    instruction at 0.5 cycles/row). The transposed fp8 operand comes from
    the xbar DMA transpose moving fp8 PAIRS as uint16 lanes straight from
    DRAM: out[p, g, q](u16) = enc-pair(e=2(128g+p)(+0/1), tok q). The pair
    interleave is absorbed by the DoubleRow k-pair dimension with a
    host-permuted weight layout w8[p, g, i, d] = 64*a_w[DEC+2(128g+p)+i, d],
    so no on-chip bf16->fp8 cast and no bf16 staging is needed.
  - e_proj PSUM is [128d, 2x512tok] (a chunk pair, 2 banks) so one tanh
    activation covers 1024 tokens per d-tile, amortizing the ~185ns
    ScalarE access overhead; bias (h_proj + a_b) is per-partition.
  - scores = v . tanh as fp8 DoubleRow over d-tile pairs (v padded to
    M=2 / k-stride 512 for the dual-fp8 ldweights ISA restriction); exp
    is fused into the PSUM evacuation (scale=1/64 undoes the *64 weight
    scaling). Softmax tail on DVE.
  - The weighted sum accumulates chunk-major into one PSUM bank
    (e-halves at partitions 0/32) so nat buffers free chunk-by-chunk,
    and each iteration emits weighted(b-1) before eproj(b) so next-row
    DMA overlaps this row's PE work.
"""

import numpy as np
from contextlib import ExitStack

B, S, ENC, DEC = 32, 2048, 1024, 1024
N_CORES = 8
BC = B // N_CORES  # batch rows per core
CH = 512           # tokens per chunk


def build_bass_kernel(bc=BC, s=S, e_dim=ENC, d_dim=DEC, debug=False):
    import concourse.bass as bass
    import concourse.tile as tile
    from concourse import bacc, mybir

    f32 = mybir.dt.float32
    bf16 = mybir.dt.bfloat16
    fp8 = mybir.dt.float8e4
    u16 = mybir.dt.uint16
    i32 = mybir.dt.int32
    Tanh = mybir.ActivationFunctionType.Tanh
    Exp = mybir.ActivationFunctionType.Exp
    DR = mybir.MatmulPerfMode.DoubleRow

    assert s % (2 * CH) == 0 and e_dim % 256 == 0 and d_dim % 256 == 0
    n_chunks = s // CH             # 512-token chunks per batch row
    n_st = CH // 128               # s-tiles per chunk
    n_g = e_dim // 256             # e pair-groups (256 e-rows per group)
    n_dt = d_dim // 128            # d (output) tiles for e_proj
    n_ec = e_dim // 512            # 512-wide e chunks for the weighted sum
    n_ct = s // 128                # s-tiles per row

    nc = bacc.Bacc("TRN2", target_bir_lowering=False, debug=debug)

    ench_h = nc.dram_tensor("ench", [bc, s, e_dim], fp8, kind="ExternalInput")
    encl_h = nc.dram_tensor("encl", [bc, s, e_dim], fp8, kind="ExternalInput")
    mskT_h = nc.dram_tensor("maskT", [bc, 128, s // 128], bf16, kind="ExternalInput")
    w8_h = nc.dram_tensor("w8", [128, n_g, 2, d_dim], fp8, kind="ExternalInput")
    wd8_h = nc.dram_tensor("wd8", [128, n_dt, d_dim], fp8, kind="ExternalInput")
    hsT8_h = nc.dram_tensor("hsT8", [128, n_dt, bc], fp8, kind="ExternalInput")
    ab_h = nc.dram_tensor("ab_t", [128, n_dt], f32, kind="ExternalInput")
    v8_h = nc.dram_tensor("v8", [128, 2, n_dt], fp8, kind="ExternalInput")
    id_h = nc.dram_tensor("ident", [128, 128], bf16, kind="ExternalInput")
    out_h = nc.dram_tensor("out", [bc, e_dim], f32, kind="ExternalOutput")

    with tile.TileContext(nc) as tc, ExitStack() as ctx:
        consts = ctx.enter_context(tc.tile_pool(name="consts", bufs=1))
        nat_pool = ctx.enter_context(tc.tile_pool(name="nat", bufs=3 * n_chunks))
        eT_pool = ctx.enter_context(tc.tile_pool(name="eT", bufs=3 * n_chunks + 1))
        th_pool = ctx.enter_context(tc.tile_pool(name="th", bufs=2))
        sm_pool = ctx.enter_context(tc.tile_pool(name="softmax", bufs=3))
        small_pool = ctx.enter_context(tc.tile_pool(name="small", bufs=4))
        outsb_pool = ctx.enter_context(tc.tile_pool(name="outsb", bufs=1))
        pe_psum = ctx.enter_context(tc.tile_pool(name="pe_psum", bufs=2, space="PSUM"))
        sc_psum = ctx.enter_context(tc.tile_pool(name="sc_psum", bufs=2, space="PSUM"))
        w_psum = ctx.enter_context(tc.tile_pool(name="w_psum", bufs=1, space="PSUM"))

        # ---------------- consts ----------------
        ident_sb = consts.tile([128, 128], bf16)
        ones_bf = ident_sb[0:1, 0:1]
        ones_f = consts.tile([128, 1], f32)
        nc.vector.memset(ones_f, 1.0)

        w8_sb = consts.tile([128, n_g, 2, d_dim], fp8)
        # dual-fp8 ldweights needs a wide stride between the k-pair weight
        # blocks (walrus s3_lw_dual_fp8_restrictions rejects stride 2/4;
        # 512 verified on HW) -> stage v into a padded tile
        v8_sb = consts.tile([128, 2, 512], fp8)
        ab_sb = consts.tile([128, n_dt], f32)
        hsT8_sb = consts.tile([128, n_dt, bc], fp8)
        wd8_sb = consts.tile([128, n_dt, d_dim], fp8)

        def emit_consts(step):
            # wd8 lands before w8: the h_proj -> hb chain completes while
            # the PE is still waiting for w8 + the first transposes, so the
            # first tanh is never gated on hb
            if step == 0:
                nc.sync.dma_start(out=wd8_sb, in_=wd8_h[:, :, :])
                nc.sync.dma_start(out=hsT8_sb, in_=hsT8_h[:, :, :])
                nc.sync.dma_start(out=ab_sb, in_=ab_h[:, :])
            elif step == 1:
                nc.sync.dma_start(out=w8_sb, in_=w8_h[:, :, :, :])
                nc.sync.dma_start(out=v8_sb[:, :, 0 : n_dt], in_=v8_h[:, :, :])
                nc.sync.dma_start(out=ident_sb, in_=id_h[:, :])

        hb_sb = consts.tile([128, n_dt, bc], f32)

        state = {}

        def emit_xbar_chunk(b, c):
            # transpose fp8 pairs (as u16 lanes) straight from DRAM, one
            # 512-row instruction per chunk:
            # out[p, g, q] = enc8u[b, CH*c + q, 128*g + p]
            eT = eT_pool.tile([128, n_g, CH], u16, tag="eT")
            nc.sync.dma_start(
                out=eT,
                in_=ench_h[b, CH * c : CH * (c + 1), :].bitcast(u16),
                transpose=True,
            )
            state[(b, c)] = dict(eT=eT)

        def emit_load_chunk(b, c):
            nath = nat_pool.tile([128, n_st, e_dim], fp8, tag="nath")
            nc.sync.dma_start(
                out=nath,
                in_=ench_h[b, CH * c : CH * (c + 1), :].rearrange(
                    "(j p) e -> p j e", p=128
                ),
            )
            natl = nat_pool.tile([128, n_st, e_dim], fp8, tag="natl")
            nc.sync.dma_start(
                out=natl,
                in_=encl_h[b, CH * c : CH * (c + 1), :].rearrange(
                    "(j p) e -> p j e", p=128
                ),
            )
            state[(b, c)]["nath"] = nath
            state[(b, c)]["natl"] = natl

        def emit_hproj():
            hp = sc_psum.tile([128, n_dt, bc], f32, tag="sc")
            for i in range(n_dt):
                for u in range(n_dt // 2):
                    nc.tensor.matmul(
                        hp[:, i, :],
                        lhsT=wd8_sb[:, 2 * u : 2 * u + 2, 128 * i : 128 * (i + 1)],
                        rhs=hsT8_sb[:, 2 * u : 2 * u + 2, :],
                        start=(u == 0),
                        stop=(u == n_dt // 2 - 1),
                        perf_mode=DR,
                    )
            for i in range(n_dt):
                nc.vector.tensor_scalar(
                    hb_sb[:, i, :], hp[:, i, :], 1.0 / 64,
                    ab_sb[:, i : i + 1],
                    op0=mybir.AluOpType.mult, op1=mybir.AluOpType.add,
                )

        def emit_eproj_pair(b, cp, mid_hook=None):
            eT = []
            for c in (2 * cp, 2 * cp + 1):
                # [p, g, q](u16) -> fp8 [p, g, (q two)]; per (j, g) the
                # DoubleRow rhs is [p, two, q]
                eT.append(state[(b, c)]["eT"][:, :, :].bitcast(fp8))
            th = th_pool.tile([128, n_dt, 2 * CH], fp8, tag="th")
            for i in range(n_dt):
                ps = pe_psum.tile([128, 2, CH], f32, tag="pe")
                for h in range(2):
                    for j in range(n_st):
                        for g in range(n_g):
                            rhs = eT[h][:, g, 256 * j : 256 * (j + 1)].rearrange(
                                "p (q two) -> p two q", two=2
                            )
                            nc.tensor.matmul(
                                ps[:, h, 128 * j : 128 * (j + 1)],
                                lhsT=w8_sb[:, g, :, 128 * i : 128 * (i + 1)],
                                rhs=rhs,
                                start=(g == 0),
                                stop=(g == n_g - 1),
                                perf_mode=DR,
                            )
                if mid_hook is not None:
                    # h_proj needs to land before the first tanh reads hb
                    # (program-order RAW), but after d-tile 0's matmuls so
                    # the PE ramps on e_proj while w_dec arrives
                    mid_hook()
                    mid_hook = None
                nc.scalar.activation(
                    th[:, i, :], ps, Tanh, bias=hb_sb[:, i, b : b + 1], scale=1.0 / 64
                )
            state[(b, cp, "th")] = th

        def emit_scores(b, cp):
            th = state[(b, cp, "th")]
            scores = state[b]["scores"]
            for h in range(2):
                sc = sc_psum.tile([2, CH], f32, tag="sc")
                for m in range(n_dt // 2):
                    nc.tensor.matmul(
                        sc,
                        lhsT=v8_sb[:, :, 2 * m : 2 * m + 2],
                        rhs=th[:, 2 * m : 2 * m + 2, CH * h : CH * (h + 1)],
                        start=(m == 0),
                        stop=(m == n_dt // 2 - 1),
                        perf_mode=DR,
                    )
                pos = CH * (2 * cp + h)
                nc.scalar.activation(
                    scores[:, pos : pos + CH], sc[0:1, :], Exp, bias=0.0, scale=1.0 / 64
                )

        def emit_row_prep(b):
            maskT = sm_pool.tile([128, n_ct], bf16, tag="maskT")
            nc.sync.dma_start(out=maskT, in_=mskT_h[b, :, :])
            scores = sm_pool.tile([1, s], bf16, tag="scores")
            state[b] = dict(maskT=maskT, scores=scores)

        def emit_weighted_part(b, cp):
            # transpose this pair's exp(scores) into columns, apply the mask
            # during psum evacuation, then accumulate the (unnormalized)
            # weighted sum for the pair's two chunks. The 1/sum normalizer is
            # folded into the final output evacuation, so none of this waits
            # on a full-row softmax.
            scores = state[b]["scores"]
            maskT = state[b]["maskT"]
            half = n_ct // 2
            j0 = cp * half
            psum_at = sc_psum.tile([128, half], f32, tag="sc")
            for j in range(half):
                nc.tensor.matmul(
                    psum_at[:, j : j + 1],
                    lhsT=scores[:, 128 * (j0 + j) : 128 * (j0 + j + 1)],
                    rhs=ones_bf,
                    start=True,
                    stop=True,
                )
            if cp == 0:
                attnT_new = small_pool.tile([128, n_ct], bf16, tag="attnT")
                ah_new = small_pool.tile([128, 2, 512], fp8, tag="ah")
                al_new = small_pool.tile([128, 2, 512], fp8, tag="al")
                pw_new = w_psum.tile([1, 2, 512], f32, tag="w")
                state[b]["attnT"] = attnT_new
                state[b]["ah"] = ah_new
                state[b]["al"] = al_new
                state[b]["pw"] = pw_new
            attnT = state[b]["attnT"]
            ah, al = state[b]["ah"], state[b]["al"]
            pw = state[b]["pw"]
            nc.vector.tensor_mul(
                attnT[:, j0 : j0 + half], psum_at, maskT[:, j0 : j0 + half]
            )
            # split attn into fp8 + fp8 residual, packed by s-tile-pair parity
            # with a 512 k-stride (dual-fp8 ldweights wants wide strides);
            # ah[p, i, u] = attn(tok=(2u+i)*128+p)
            u0 = half // 2 * cp
            nu = half // 2
            asrc = attnT[:, j0 : j0 + half].rearrange("p (u two) -> p two u", two=2)
            nc.vector.tensor_copy(out=ah[:, :, u0 : u0 + nu], in_=asrc)
            nc.vector.tensor_tensor(
                out=al[:, :, u0 : u0 + nu], in0=asrc, in1=ah[:, :, u0 : u0 + nu],
                op=mybir.AluOpType.subtract,
            )
            # hi*hi + hi*lo + lo*hi accumulate into one psum group
            # (residuals are unscaled fp8, so no rescale is needed)
            first_u, last_u = n_ct // 4 * cp, n_ct // 4 * (cp + 1) - 1
            for ec in range(n_ec):
                for u in range(first_u, last_u + 1):
                    c, jj = divmod(2 * u, n_st)
                    nath = state[(b, c)]["nath"][:, jj : jj + 2, 512 * ec : 512 * (ec + 1)]
                    natl = state[(b, c)]["natl"][:, jj : jj + 2, 512 * ec : 512 * (ec + 1)]
                    for src_a, src_e, is_first, is_last in (
                        (ah, nath, cp == 0 and u == first_u, False),
                        (ah, natl, False, False),
                        (al, nath, False, cp == n_chunks // 2 - 1 and u == last_u),
                    ):
                        nc.tensor.matmul(
                            pw[:, ec, :],
                            lhsT=src_a[:, :, u : u + 1],
                            rhs=src_e,
                            start=is_first,
                            stop=is_last,
                            perf_mode=DR,
                        )

        def emit_weighted_finish(b):
            attnT = state[b]["attnT"]
            pw = state[b]["pw"]
            partials = small_pool.tile([128, 1], f32, tag="part")
            nc.vector.reduce_sum(out=partials, in_=attnT, axis=mybir.AxisListType.X)
            psum_s = sc_psum.tile([1, 1], f32, tag="sc")
            nc.tensor.matmul(
                psum_s, lhsT=partials, rhs=ones_f, start=True, stop=True
            )
            rsum = small_pool.tile([1, 1], f32, tag="rsum")
            nc.vector.reciprocal(rsum, psum_s)
            out_sb = outsb_pool.tile([1, e_dim], f32, tag="outsb")
            for ec in range(n_ec):
                nc.vector.tensor_scalar_mul(
                    out_sb[:, 512 * ec : 512 * (ec + 1)],
                    pw[:, ec, :],
                    rsum[0:1, 0:1],
                )
            nc.sync.dma_start(out=out_h[b : b + 1, :], in_=out_sb)

        # ---------------- schedule ----------------
        emit_xbar_chunk(0, 0)
        emit_xbar_chunk(0, 1)
        emit_consts(0)
        emit_consts(1)
        emit_hproj()
        emit_xbar_chunk(0, 2)
        emit_xbar_chunk(0, 3)
        emit_row_prep(0)
        for c in range(n_chunks):
            emit_load_chunk(0, c)
        # one-pair stagger across the whole pipeline: while the PE runs
        # eproj of pair P, it then retires scores/exp/transpose/weighted of
        # pair P-1, whose Activation-side work completed during eproj(P) --
        # the PE never waits on ScalarE.
        pairs = [(b, cp) for b in range(bc) for cp in range(n_chunks // 2)]
        for idx, (b, cp) in enumerate(pairs):
            emit_eproj_pair(b, cp)
            if idx >= 1:
                pb, pcp = pairs[idx - 1]
                emit_scores(pb, pcp)
                emit_weighted_part(pb, pcp)
                if pcp == n_chunks // 2 - 1:
                    emit_weighted_finish(pb)
            # loads come AFTER the retirement above (its weighted matmuls
            # free the nat ring slots these loads reuse), batched two rows
            # at a time so the copy<->transpose queue-mode switch drains
            # happen half as often
            if cp == 0:
                next_rows = [r for r in (
                    (b + 1, b + 2) if b % 2 == 0 else ()
                ) if r < bc]
                for r in next_rows:
                    for c in range(n_chunks):
                        emit_xbar_chunk(r, c)
                for r in next_rows:
                    emit_row_prep(r)
                    for c in range(n_chunks):
                        emit_load_chunk(r, c)
        pb, pcp = pairs[-1]
        emit_scores(pb, pcp)
        emit_weighted_part(pb, pcp)
        emit_weighted_finish(pb)

    nc.compile()
    return nc


_CACHE = {}


def _prep_weights(a_w, a_b, v_w, e_dim=ENC, d_dim=DEC):
    import ml_dtypes

    fp8 = ml_dtypes.float8_e4m3
    n_g, n_dt = e_dim // 256, d_dim // 128
    # w8[p, g, i, d] = 64 * a_w[DEC + 2*(128*g + p) + i, d]
    w8 = (
        (np.asarray(a_w[d_dim:], np.float32) * 64.0)
        .reshape(n_g, 128, 2, d_dim).transpose(1, 0, 2, 3).astype(fp8)
    )
    wd8 = (
        (np.asarray(a_w[:d_dim], np.float32) * 64.0)
        .reshape(n_dt, 128, d_dim).transpose(1, 0, 2).astype(fp8)
    )
    # v8[p, i, 2m+r] = 64 * v_w[(2m+i)*128 + p]  (duplicated along r: the
    # dual-fp8 ldweights wants M=2 columns)
    v8 = np.repeat(
        (np.asarray(v_w, np.float32) * 64.0)
        .reshape(n_dt // 2, 2, 128).transpose(2, 1, 0).astype(fp8)[:, :, :, None],
        2, axis=3,
    ).reshape(128, 2, n_dt)
    ab_t = np.ascontiguousarray(
        np.asarray(a_b, np.float32).reshape(n_dt, 128).T
    )
    return (
        np.ascontiguousarray(w8),
        np.ascontiguousarray(wd8),
        np.ascontiguousarray(v8),
        ab_t,
    )


def kernel(hidden_states, encoder_outputs, encoder_masks, a_w, a_b, v_w):
    import ml_dtypes
    from concourse.bass_utils import run_bass_kernel_spmd

    if "nc" not in _CACHE:
        _CACHE["nc"] = build_bass_kernel()
    nc = _CACHE["nc"]

    bf16 = ml_dtypes.bfloat16
    fp8 = ml_dtypes.float8_e4m3
    hidden_states = np.asarray(hidden_states, dtype=np.float32)
    enc_f32 = np.asarray(encoder_outputs, dtype=np.float32)
    ench = enc_f32.astype(fp8)
    encl = (enc_f32 - ench.astype(np.float32)).astype(fp8)
    encoder_masks = np.asarray(encoder_masks, dtype=np.int32)
    w8, wd8, v8, ab_t = _prep_weights(a_w, a_b, v_w)
    ident = np.eye(128, dtype=bf16)
    n_dt = DEC // 128

    in_maps = []
    for c in range(N_CORES):
        sl = slice(c * BC, (c + 1) * BC)
        hsT8 = np.ascontiguousarray(
            hidden_states[sl].T.reshape(n_dt, 128, BC).transpose(1, 0, 2)
        ).astype(fp8)
        maskT = np.ascontiguousarray(
            (encoder_masks[sl] != 0)
            .reshape(BC, S // 128, 128).transpose(0, 2, 1)
        ).astype(bf16)
        m = {
            "ench": np.ascontiguousarray(ench[sl]),
            "encl": np.ascontiguousarray(encl[sl]),
            "maskT": maskT,
            "w8": w8,
            "wd8": wd8,
            "hsT8": np.ascontiguousarray(hsT8),
            "ab_t": ab_t,
            "v8": v8,
            "ident": ident,
        }
        in_maps.append(m)

    global _LAST_IN_MAPS
    _LAST_IN_MAPS = in_maps
    res = run_bass_kernel_spmd(nc, in_maps, core_ids=list(range(N_CORES)))
    out = np.concatenate([r["out"] for r in res.results], axis=0)
    return out.astype(np.float32)


_LAST_IN_MAPS = None


# revision 35
# speedup vs baseline: 2.7548x; 1.0132x over previous
"""Bahdanau-style attention kernel for Trainium2 (8 NeuronCores, SPMD).

Math (per batch row b):
    h_proj = hidden @ a_w[:DEC]                       (DEC,)
    e_proj[s, :] = enc[s, :] @ a_w[DEC:]              (S, DEC)
    energy = tanh(e_proj + h_proj + a_b)              (S, DEC)
    scores = energy @ v_w                             (S,)
    scores = where(mask == 0, -1e10, scores)
    attn = softmax(scores)                            (S,)
    out = attn @ enc                                  (ENC,)

Sharding: data-parallel over batch (32 rows -> 4 rows on each of 8 cores);
weights replicated (pre-quantized to fp8*64 on host).

Per-core strategy:
  - The weighted sum runs in bf16 from natural-layout [tok, e] chunks
    (host-cast enc); softmax-averaging keeps per-element quantization
    error in the output, so fp8 enc there would blow the 2e-2 gate.
  - e_proj runs in fp8 with MatmulPerfMode.DoubleRow (2 k-tiles per
    instruction at 0.5 cycles/row). The transposed fp8 operand comes from
    the xbar DMA transpose moving fp8 PAIRS as uint16 lanes straight from
    DRAM: out[p, g, q](u16) = enc-pair(e=2(128g+p)(+0/1), tok q). The pair
    interleave is absorbed by the DoubleRow k-pair dimension with a
    host-permuted weight layout w8[p, g, i, d] = 64*a_w[DEC+2(128g+p)+i, d],
    so no on-chip bf16->fp8 cast and no bf16 staging is needed.
  - e_proj PSUM is [128d, 2x512tok] (a chunk pair, 2 banks) so one tanh
    activation covers 1024 tokens per d-tile, amortizing the ~185ns
    ScalarE access overhead; bias (h_proj + a_b) is per-partition.
  - scores = v . tanh as fp8 DoubleRow over d-tile pairs (v padded to
    M=2 / k-stride 512 for the dual-fp8 ldweights ISA restriction); exp
    is fused into the PSUM evacuation (scale=1/64 undoes the *64 weight
    scaling). Softmax tail on DVE.
  - The weighted sum accumulates chunk-major into one PSUM bank
    (e-halves at partitions 0/32) so nat buffers free chunk-by-chunk,
    and each iteration emits weighted(b-1) before eproj(b) so next-row
    DMA overlaps this row's PE work.
"""

import numpy as np
from contextlib import ExitStack

B, S, ENC, DEC = 32, 2048, 1024, 1024
N_CORES = 8
BC = B // N_CORES  # batch rows per core
CH = 512           # tokens per chunk


def build_bass_kernel(bc=BC, s=S, e_dim=ENC, d_dim=DEC, debug=False):
    import concourse.bass as bass
    import concourse.tile as tile
    from concourse import bacc, mybir

    f32 = mybir.dt.float32
    bf16 = mybir.dt.bfloat16
    fp8 = mybir.dt.float8e4
    u16 = mybir.dt.uint16
    i32 = mybir.dt.int32
    Tanh = mybir.ActivationFunctionType.Tanh
    Exp = mybir.ActivationFunctionType.Exp
    DR = mybir.MatmulPerfMode.DoubleRow

    assert s % (2 * CH) == 0 and e_dim % 256 == 0 and d_dim % 256 == 0
    n_chunks = s // CH             # 512-token chunks per batch row
    n_st = CH // 128               # s-tiles per chunk
    n_g = e_dim // 256             # e pair-groups (256 e-rows per group)
    n_dt = d_dim // 128            # d (output) tiles for e_proj
    n_ec = e_dim // 512            # 512-wide e chunks for the weighted sum
    n_ct = s // 128                # s-tiles per row

    nc = bacc.Bacc("TRN2", target_bir_lowering=False, debug=debug)

    ench_h = nc.dram_tensor("ench", [bc, s, e_dim], fp8, kind="ExternalInput")
    encl_h = nc.dram_tensor("encl", [bc, s, e_dim], fp8, kind="ExternalInput")
    mskT_h = nc.dram_tensor("maskT", [bc, 128, s // 128], bf16, kind="ExternalInput")
    w8_h = nc.dram_tensor("w8", [128, n_g, 2, d_dim], fp8, kind="ExternalInput")
    wd8_h = nc.dram_tensor("wd8", [128, n_dt, d_dim], fp8, kind="ExternalInput")
    hsT8_h = nc.dram_tensor("hsT8", [128, n_dt, bc], fp8, kind="ExternalInput")
    ab_h = nc.dram_tensor("ab_t", [128, n_dt], f32, kind="ExternalInput")
    v8_h = nc.dram_tensor("v8", [128, 2, n_dt], fp8, kind="ExternalInput")
    id_h = nc.dram_tensor("ident", [128, 128], bf16, kind="ExternalInput")
    out_h = nc.dram_tensor("out", [bc, e_dim], f32, kind="ExternalOutput")

    with tile.TileContext(nc) as tc, ExitStack() as ctx:
        consts = ctx.enter_context(tc.tile_pool(name="consts", bufs=1))
        nat_pool = ctx.enter_context(tc.tile_pool(name="nat", bufs=3 * n_chunks))
        eT_pool = ctx.enter_context(tc.tile_pool(name="eT", bufs=3 * n_chunks + 1))
        th_pool = ctx.enter_context(tc.tile_pool(name="th", bufs=2))
        sm_pool = ctx.enter_context(tc.tile_pool(name="softmax", bufs=3))
        small_pool = ctx.enter_context(tc.tile_pool(name="small", bufs=4))
        outsb_pool = ctx.enter_context(tc.tile_pool(name="outsb", bufs=1))
        pe_psum = ctx.enter_context(tc.tile_pool(name="pe_psum", bufs=2, space="PSUM"))
        sc_psum = ctx.enter_context(tc.tile_pool(name="sc_psum", bufs=2, space="PSUM"))
        w_psum = ctx.enter_context(tc.tile_pool(name="w_psum", bufs=1, space="PSUM"))

        # ---------------- consts ----------------
        ident_sb = consts.tile([128, 128], bf16)
        ones_bf = ident_sb[0:1, 0:1]
        ones_f = consts.tile([128, 1], f32)
        nc.vector.memset(ones_f, 1.0)

        w8_sb = consts.tile([128, n_g, 2, d_dim], fp8)
        # dual-fp8 ldweights needs a wide stride between the k-pair weight
        # blocks (walrus s3_lw_dual_fp8_restrictions rejects stride 2/4;
        # 512 verified on HW) -> stage v into a padded tile
        v8_sb = consts.tile([128, 2, 512], fp8)
        ab_sb = consts.tile([128, n_dt], f32)
        hsT8_sb = consts.tile([128, n_dt, bc], fp8)
        wd8_sb = consts.tile([128, n_dt, d_dim], fp8)

        def emit_consts(step):
            # wd8 lands before w8: the h_proj -> hb chain completes while
            # the PE is still waiting for w8 + the first transposes, so the
            # first tanh is never gated on hb
            if step == 0:
                nc.sync.dma_start(out=wd8_sb, in_=wd8_h[:, :, :])
                nc.sync.dma_start(out=hsT8_sb, in_=hsT8_h[:, :, :])
                nc.sync.dma_start(out=ab_sb, in_=ab_h[:, :])
            elif step == 1:
                nc.sync.dma_start(out=w8_sb, in_=w8_h[:, :, :, :])
                nc.sync.dma_start(out=v8_sb[:, :, 0 : n_dt], in_=v8_h[:, :, :])
                nc.sync.dma_start(out=ident_sb, in_=id_h[:, :])

        hb_sb = consts.tile([128, n_dt, bc], f32)
        warm_sb = consts.tile([1, 1], f32)

        state = {}

        def emit_warmup(n_tr):
            # pull the activation-table load off the first-tanh critical path
            nc.scalar.activation(warm_sb, ones_f[0:1, 0:1], Tanh, bias=0.0, scale=1.0)
            # keep the PE busy until w8 lands so e_proj starts at full
            # p-state (the cost model halves PE speed for the first 3us
            # after an idle period)
            for k in range(n_tr):
                pswarm = sc_psum.tile([128, 128], f32, tag="sc")
                nc.tensor.matmul(
                    pswarm, lhsT=wd8_sb[:, 0:2, 0:128],
                    rhs=wd8_sb[:, 0:2, 0:128],
                    start=True, stop=True,
                    perf_mode=DR,
                )

        def emit_xbar_chunk(b, c):
            # transpose fp8 pairs (as u16 lanes) straight from DRAM, one
            # 512-row instruction per chunk:
            # out[p, g, q] = enc8u[b, CH*c + q, 128*g + p]
            eT = eT_pool.tile([128, n_g, CH], u16, tag="eT")
            nc.sync.dma_start(
                out=eT,
                in_=ench_h[b, CH * c : CH * (c + 1), :].bitcast(u16),
                transpose=True,
            )
            state[(b, c)] = dict(eT=eT)

        def emit_load_chunk(b, c):
            nath = nat_pool.tile([128, n_st, e_dim], fp8, tag="nath")
            nc.sync.dma_start(
                out=nath,
                in_=ench_h[b, CH * c : CH * (c + 1), :].rearrange(
                    "(j p) e -> p j e", p=128
                ),
            )
            natl = nat_pool.tile([128, n_st, e_dim], fp8, tag="natl")
            nc.sync.dma_start(
                out=natl,
                in_=encl_h[b, CH * c : CH * (c + 1), :].rearrange(
                    "(j p) e -> p j e", p=128
                ),
            )
            state[(b, c)]["nath"] = nath
            state[(b, c)]["natl"] = natl

        def emit_hproj():
            hp = sc_psum.tile([128, n_dt, bc], f32, tag="sc")
            for i in range(n_dt):
                for u in range(n_dt // 2):
                    nc.tensor.matmul(
                        hp[:, i, :],
                        lhsT=wd8_sb[:, 2 * u : 2 * u + 2, 128 * i : 128 * (i + 1)],
                        rhs=hsT8_sb[:, 2 * u : 2 * u + 2, :],
                        start=(u == 0),
                        stop=(u == n_dt // 2 - 1),
                        perf_mode=DR,
                    )
            for i in range(n_dt):
                nc.vector.tensor_scalar(
                    hb_sb[:, i, :], hp[:, i, :], 1.0 / 64,
                    ab_sb[:, i : i + 1],
                    op0=mybir.AluOpType.mult, op1=mybir.AluOpType.add,
                )

        def emit_eproj_pair(b, cp, mid_hook=None):
            eT = []
            for c in (2 * cp, 2 * cp + 1):
                # [p, g, q](u16) -> fp8 [p, g, (q two)]; per (j, g) the
                # DoubleRow rhs is [p, two, q]
                eT.append(state[(b, c)]["eT"][:, :, :].bitcast(fp8))
            th = th_pool.tile([128, n_dt, 2 * CH], fp8, tag="th")
            for i in range(n_dt):
                ps = pe_psum.tile([128, 2, CH], f32, tag="pe")
                for h in range(2):
                    for j in range(n_st):
                        for g in range(n_g):
                            rhs = eT[h][:, g, 256 * j : 256 * (j + 1)].rearrange(
                                "p (q two) -> p two q", two=2
                            )
                            nc.tensor.matmul(
                                ps[:, h, 128 * j : 128 * (j + 1)],
                                lhsT=w8_sb[:, g, :, 128 * i : 128 * (i + 1)],
                                rhs=rhs,
                                start=(g == 0),
                                stop=(g == n_g - 1),
                                perf_mode=DR,
                            )
                if mid_hook is not None:
                    # h_proj needs to land before the first tanh reads hb
                    # (program-order RAW), but after d-tile 0's matmuls so
                    # the PE ramps on e_proj while w_dec arrives
                    mid_hook()
                    mid_hook = None
                nc.scalar.activation(
                    th[:, i, :], ps, Tanh, bias=hb_sb[:, i, b : b + 1], scale=1.0 / 64
                )
            state[(b, cp, "th")] = th

        def emit_scores(b, cp):
            th = state[(b, cp, "th")]
            scores = state[b]["scores"]
            for h in range(2):
                sc = sc_psum.tile([2, CH], f32, tag="sc")
                for m in range(n_dt // 2):
                    nc.tensor.matmul(
                        sc,
                        lhsT=v8_sb[:, :, 2 * m : 2 * m + 2],
                        rhs=th[:, 2 * m : 2 * m + 2, CH * h : CH * (h + 1)],
                        start=(m == 0),
                        stop=(m == n_dt // 2 - 1),
                        perf_mode=DR,
                    )
                pos = CH * (2 * cp + h)
                nc.scalar.activation(
                    scores[:, pos : pos + CH], sc[0:1, :], Exp, bias=0.0, scale=1.0 / 64
                )

        def emit_row_prep(b):
            maskT = sm_pool.tile([128, n_ct], bf16, tag="maskT")
            nc.sync.dma_start(out=maskT, in_=mskT_h[b, :, :])
            scores = sm_pool.tile([1, s], bf16, tag="scores")
            state[b] = dict(maskT=maskT, scores=scores)

        def emit_weighted_part(b, cp):
            # transpose this pair's exp(scores) into columns, apply the mask
            # during psum evacuation, then accumulate the (unnormalized)
            # weighted sum for the pair's two chunks. The 1/sum normalizer is
            # folded into the final output evacuation, so none of this waits
            # on a full-row softmax.
            scores = state[b]["scores"]
            maskT = state[b]["maskT"]
            half = n_ct // 2
            j0 = cp * half
            psum_at = sc_psum.tile([128, half], f32, tag="sc")
            for j in range(half):
                nc.tensor.matmul(
                    psum_at[:, j : j + 1],
                    lhsT=scores[:, 128 * (j0 + j) : 128 * (j0 + j + 1)],
                    rhs=ones_bf,
                    start=True,
                    stop=True,
                )
            if cp == 0:
                attnT_new = small_pool.tile([128, n_ct], bf16, tag="attnT")
                ah_new = small_pool.tile([128, 2, 512], fp8, tag="ah")
                al_new = small_pool.tile([128, 2, 512], fp8, tag="al")
                pw_new = w_psum.tile([1, 2, 512], f32, tag="w")
                state[b]["attnT"] = attnT_new
                state[b]["ah"] = ah_new
                state[b]["al"] = al_new
                state[b]["pw"] = pw_new
            attnT = state[b]["attnT"]
            ah, al = state[b]["ah"], state[b]["al"]
            pw = state[b]["pw"]
            nc.vector.tensor_mul(
                attnT[:, j0 : j0 + half], psum_at, maskT[:, j0 : j0 + half]
            )
            # split attn into fp8 + fp8 residual, packed by s-tile-pair parity
            # with a 512 k-stride (dual-fp8 ldweights wants wide strides);
            # ah[p, i, u] = attn(tok=(2u+i)*128+p)
            u0 = half // 2 * cp
            nu = half // 2
            asrc = attnT[:, j0 : j0 + half].rearrange("p (u two) -> p two u", two=2)
            nc.vector.tensor_copy(out=ah[:, :, u0 : u0 + nu], in_=asrc)
            nc.vector.tensor_tensor(
                out=al[:, :, u0 : u0 + nu], in0=asrc, in1=ah[:, :, u0 : u0 + nu],
                op=mybir.AluOpType.subtract,
            )
            if cp == n_chunks // 2 - 1:
                # row sum + reciprocal ahead of the weighted matmuls: the
                # tiny sum matmul would otherwise queue behind them on the
                # PE, delaying the final evacuation by the whole pair
                partials = small_pool.tile([128, 1], f32, tag="part")
                nc.vector.reduce_sum(
                    out=partials, in_=attnT, axis=mybir.AxisListType.X
                )
                psum_s = sc_psum.tile([1, 1], f32, tag="sc")
                nc.tensor.matmul(
                    psum_s, lhsT=partials, rhs=ones_f, start=True, stop=True
                )
                rsum = small_pool.tile([1, 1], f32, tag="rsum")
                nc.vector.reciprocal(rsum, psum_s)
                state[b]["rsum"] = rsum
            # hi*hi + hi*lo + lo*hi accumulate into one psum group
            # (residuals are unscaled fp8, so no rescale is needed)
            first_u, last_u = n_ct // 4 * cp, n_ct // 4 * (cp + 1) - 1
            for ec in range(n_ec):
                for u in range(first_u, last_u + 1):
                    c, jj = divmod(2 * u, n_st)
                    nath = state[(b, c)]["nath"][:, jj : jj + 2, 512 * ec : 512 * (ec + 1)]
                    natl = state[(b, c)]["natl"][:, jj : jj + 2, 512 * ec : 512 * (ec + 1)]
                    for src_a, src_e, is_first, is_last in (
                        (ah, nath, cp == 0 and u == first_u, False),
                        (ah, natl, False, False),
                        (al, nath, False, cp == n_chunks // 2 - 1 and u == last_u),
                    ):
                        nc.tensor.matmul(
                            pw[:, ec, :],
                            lhsT=src_a[:, :, u : u + 1],
                            rhs=src_e,
                            start=is_first,
                            stop=is_last,
                            perf_mode=DR,
                        )

        def emit_weighted_finish(b):
            pw = state[b]["pw"]
            rsum = state[b]["rsum"]
            out_sb = outsb_pool.tile([1, e_dim], f32, tag="outsb")
            for ec in range(n_ec):
                nc.vector.tensor_scalar_mul(
                    out_sb[:, 512 * ec : 512 * (ec + 1)],
                    pw[:, ec, :],
                    rsum[0:1, 0:1],
                )
            nc.sync.dma_start(out=out_h[b : b + 1, :], in_=out_sb)

        # ---------------- schedule ----------------
        emit_xbar_chunk(0, 0)
        emit_xbar_chunk(0, 1)
        emit_consts(0)
        emit_consts(1)
        emit_hproj()
        emit_warmup(40)
        emit_xbar_chunk(0, 2)
        emit_xbar_chunk(0, 3)
        emit_row_prep(0)
        for c in range(n_chunks):
            emit_load_chunk(0, c)
        # one-pair stagger across the whole pipeline: while the PE runs
        # eproj of pair P, it then retires scores/exp/transpose/weighted of
        # pair P-1, whose Activation-side work completed during eproj(P) --
        # the PE never waits on ScalarE.
        pairs = [(b, cp) for b in range(bc) for cp in range(n_chunks // 2)]
        for idx, (b, cp) in enumerate(pairs):
            emit_eproj_pair(b, cp)
            if idx >= 1:
                pb, pcp = pairs[idx - 1]
                emit_scores(pb, pcp)
                emit_weighted_part(pb, pcp)
                if pcp == n_chunks // 2 - 1:
                    emit_weighted_finish(pb)
            # loads come AFTER the retirement above (its weighted matmuls
            # free the nat ring slots these loads reuse), batched two rows
            # at a time so the copy<->transpose queue-mode switch drains
            # happen half as often
            if cp == 0:
                next_rows = [r for r in (
                    (b + 1, b + 2) if b % 2 == 0 else ()
                ) if r < bc]
                for r in next_rows:
                    for c in range(n_chunks):
                        emit_xbar_chunk(r, c)
                for r in next_rows:
                    emit_row_prep(r)
                    for c in range(n_chunks):
                        emit_load_chunk(r, c)
        pb, pcp = pairs[-1]
        emit_scores(pb, pcp)
        emit_weighted_part(pb, pcp)
        emit_weighted_finish(pb)

    nc.compile()
    return nc


_CACHE = {}


def _prep_weights(a_w, a_b, v_w, e_dim=ENC, d_dim=DEC):
    import ml_dtypes

    fp8 = ml_dtypes.float8_e4m3
    n_g, n_dt = e_dim // 256, d_dim // 128
    # w8[p, g, i, d] = 64 * a_w[DEC + 2*(128*g + p) + i, d]
    w8 = (
        (np.asarray(a_w[d_dim:], np.float32) * 64.0)
        .reshape(n_g, 128, 2, d_dim).transpose(1, 0, 2, 3).astype(fp8)
    )
    wd8 = (
        (np.asarray(a_w[:d_dim], np.float32) * 64.0)
        .reshape(n_dt, 128, d_dim).transpose(1, 0, 2).astype(fp8)
    )
    # v8[p, i, 2m+r] = 64 * v_w[(2m+i)*128 + p]  (duplicated along r: the
    # dual-fp8 ldweights wants M=2 columns)
    v8 = np.repeat(
        (np.asarray(v_w, np.float32) * 64.0)
        .reshape(n_dt // 2, 2, 128).transpose(2, 1, 0).astype(fp8)[:, :, :, None],
        2, axis=3,
    ).reshape(128, 2, n_dt)
    ab_t = np.ascontiguousarray(
        np.asarray(a_b, np.float32).reshape(n_dt, 128).T
    )
    return (
        np.ascontiguousarray(w8),
        np.ascontiguousarray(wd8),
        np.ascontiguousarray(v8),
        ab_t,
    )


def kernel(hidden_states, encoder_outputs, encoder_masks, a_w, a_b, v_w):
    import ml_dtypes
    from concourse.bass_utils import run_bass_kernel_spmd

    if "nc" not in _CACHE:
        _CACHE["nc"] = build_bass_kernel()
    nc = _CACHE["nc"]

    bf16 = ml_dtypes.bfloat16
    fp8 = ml_dtypes.float8_e4m3
    hidden_states = np.asarray(hidden_states, dtype=np.float32)
    enc_f32 = np.asarray(encoder_outputs, dtype=np.float32)
    ench = enc_f32.astype(fp8)
    encl = (enc_f32 - ench.astype(np.float32)).astype(fp8)
    encoder_masks = np.asarray(encoder_masks, dtype=np.int32)
    w8, wd8, v8, ab_t = _prep_weights(a_w, a_b, v_w)
    ident = np.eye(128, dtype=bf16)
    n_dt = DEC // 128

    in_maps = []
    for c in range(N_CORES):
        sl = slice(c * BC, (c + 1) * BC)
        hsT8 = np.ascontiguousarray(
            hidden_states[sl].T.reshape(n_dt, 128, BC).transpose(1, 0, 2)
        ).astype(fp8)
        maskT = np.ascontiguousarray(
            (encoder_masks[sl] != 0)
            .reshape(BC, S // 128, 128).transpose(0, 2, 1)
        ).astype(bf16)
        m = {
            "ench": np.ascontiguousarray(ench[sl]),
            "encl": np.ascontiguousarray(encl[sl]),
            "maskT": maskT,
            "w8": w8,
            "wd8": wd8,
            "hsT8": np.ascontiguousarray(hsT8),
            "ab_t": ab_t,
            "v8": v8,
            "ident": ident,
        }
        in_maps.append(m)

    global _LAST_IN_MAPS
    _LAST_IN_MAPS = in_maps
    res = run_bass_kernel_spmd(nc, in_maps, core_ids=list(range(N_CORES)))
    out = np.concatenate([r["out"] for r in res.results], axis=0)
    return out.astype(np.float32)


_LAST_IN_MAPS = None


# revision 37
# speedup vs baseline: 2.7712x; 1.0060x over previous
"""Bahdanau-style attention kernel for Trainium2 (8 NeuronCores, SPMD).

Math (per batch row b):
    h_proj = hidden @ a_w[:DEC]                       (DEC,)
    e_proj[s, :] = enc[s, :] @ a_w[DEC:]              (S, DEC)
    energy = tanh(e_proj + h_proj + a_b)              (S, DEC)
    scores = energy @ v_w                             (S,)
    scores = where(mask == 0, -1e10, scores)
    attn = softmax(scores)                            (S,)
    out = attn @ enc                                  (ENC,)

Sharding: data-parallel over batch (32 rows -> 4 rows on each of 8 cores);
weights replicated (pre-quantized to fp8*64 on host).

Per-core strategy:
  - The weighted sum runs in bf16 from natural-layout [tok, e] chunks
    (host-cast enc); softmax-averaging keeps per-element quantization
    error in the output, so fp8 enc there would blow the 2e-2 gate.
  - e_proj runs in fp8 with MatmulPerfMode.DoubleRow (2 k-tiles per
    instruction at 0.5 cycles/row). The transposed fp8 operand comes from
    the xbar DMA transpose moving fp8 PAIRS as uint16 lanes straight from
    DRAM: out[p, g, q](u16) = enc-pair(e=2(128g+p)(+0/1), tok q). The pair
    interleave is absorbed by the DoubleRow k-pair dimension with a
    host-permuted weight layout w8[p, g, i, d] = 64*a_w[DEC+2(128g+p)+i, d],
    so no on-chip bf16->fp8 cast and no bf16 staging is needed.
  - e_proj PSUM is [128d, 2x512tok] (a chunk pair, 2 banks) so one tanh
    activation covers 1024 tokens per d-tile, amortizing the ~185ns
    ScalarE access overhead; bias (h_proj + a_b) is per-partition.
  - scores = v . tanh as fp8 DoubleRow over d-tile pairs (v padded to
    M=2 / k-stride 512 for the dual-fp8 ldweights ISA restriction); exp
    is fused into the PSUM evacuation (scale=1/64 undoes the *64 weight
    scaling). Softmax tail on DVE.
  - The weighted sum accumulates chunk-major into one PSUM bank
    (e-halves at partitions 0/32) so nat buffers free chunk-by-chunk,
    and each iteration emits weighted(b-1) before eproj(b) so next-row
    DMA overlaps this row's PE work.
"""

import numpy as np
from contextlib import ExitStack

B, S, ENC, DEC = 32, 2048, 1024, 1024
N_CORES = 8
BC = B // N_CORES  # batch rows per core
CH = 512           # tokens per chunk


def build_bass_kernel(bc=BC, s=S, e_dim=ENC, d_dim=DEC, debug=False):
    import concourse.bass as bass
    import concourse.tile as tile
    from concourse import bacc, mybir

    f32 = mybir.dt.float32
    bf16 = mybir.dt.bfloat16
    fp8 = mybir.dt.float8e4
    u16 = mybir.dt.uint16
    i32 = mybir.dt.int32
    Tanh = mybir.ActivationFunctionType.Tanh
    Exp = mybir.ActivationFunctionType.Exp
    DR = mybir.MatmulPerfMode.DoubleRow

    assert s % (2 * CH) == 0 and e_dim % 256 == 0 and d_dim % 256 == 0
    n_chunks = s // CH             # 512-token chunks per batch row
    n_st = CH // 128               # s-tiles per chunk
    n_g = e_dim // 256             # e pair-groups (256 e-rows per group)
    n_dt = d_dim // 128            # d (output) tiles for e_proj
    n_ec = e_dim // 512            # 512-wide e chunks for the weighted sum
    n_ct = s // 128                # s-tiles per row

    nc = bacc.Bacc("TRN2", target_bir_lowering=False, debug=debug)

    ench_h = nc.dram_tensor("ench", [bc, s, e_dim], fp8, kind="ExternalInput")
    encl_h = nc.dram_tensor("encl", [bc, s, e_dim], fp8, kind="ExternalInput")
    mskT_h = nc.dram_tensor("maskT", [bc, 128, s // 128], bf16, kind="ExternalInput")
    w8_h = nc.dram_tensor("w8", [128, n_g, 2, d_dim], fp8, kind="ExternalInput")
    wd8_h = nc.dram_tensor("wd8", [128, n_dt, d_dim], fp8, kind="ExternalInput")
    hsT8_h = nc.dram_tensor("hsT8", [128, n_dt, bc], fp8, kind="ExternalInput")
    ab_h = nc.dram_tensor("ab_t", [128, n_dt], f32, kind="ExternalInput")
    v8_h = nc.dram_tensor("v8", [128, 2, n_dt], fp8, kind="ExternalInput")
    id_h = nc.dram_tensor("ident", [128, 128], bf16, kind="ExternalInput")
    out_h = nc.dram_tensor("out", [bc, e_dim], f32, kind="ExternalOutput")

    with tile.TileContext(nc) as tc, ExitStack() as ctx:
        consts = ctx.enter_context(tc.tile_pool(name="consts", bufs=1))
        nat_pool = ctx.enter_context(tc.tile_pool(name="nat", bufs=3 * n_chunks))
        eT_pool = ctx.enter_context(tc.tile_pool(name="eT", bufs=3 * n_chunks + 1))
        th_pool = ctx.enter_context(tc.tile_pool(name="th", bufs=2))
        sm_pool = ctx.enter_context(tc.tile_pool(name="softmax", bufs=3))
        small_pool = ctx.enter_context(tc.tile_pool(name="small", bufs=4))
        outsb_pool = ctx.enter_context(tc.tile_pool(name="outsb", bufs=1))
        pe_psum = ctx.enter_context(tc.tile_pool(name="pe_psum", bufs=2, space="PSUM"))
        sc_psum = ctx.enter_context(tc.tile_pool(name="sc_psum", bufs=2, space="PSUM"))
        w_psum = ctx.enter_context(tc.tile_pool(name="w_psum", bufs=1, space="PSUM"))

        # ---------------- consts ----------------
        ident_sb = consts.tile([128, 128], bf16)
        ones_bf = ident_sb[0:1, 0:1]
        ones_f = consts.tile([128, 1], f32)
        nc.vector.memset(ones_f, 1.0)

        w8_sb = consts.tile([128, n_g, 2, d_dim], fp8)
        # dual-fp8 ldweights needs a wide stride between the k-pair weight
        # blocks (walrus s3_lw_dual_fp8_restrictions rejects stride 2/4;
        # 512 verified on HW) -> stage v into a padded tile
        v8_sb = consts.tile([128, 2, 512], fp8)
        ab_sb = consts.tile([128, n_dt], f32)
        hsT8_sb = consts.tile([128, n_dt, bc], fp8)
        wd8_sb = consts.tile([128, n_dt, d_dim], fp8)

        def emit_consts(step):
            # wd8 lands before w8: the h_proj -> hb chain completes while
            # the PE is still waiting for w8 + the first transposes, so the
            # first tanh is never gated on hb
            if step == 0:
                nc.sync.dma_start(out=wd8_sb, in_=wd8_h[:, :, :])
                nc.sync.dma_start(out=hsT8_sb, in_=hsT8_h[:, :, :])
                nc.sync.dma_start(out=ab_sb, in_=ab_h[:, :])
            elif step == 1:
                nc.sync.dma_start(out=w8_sb, in_=w8_h[:, :, :, :])
                nc.sync.dma_start(out=v8_sb[:, :, 0 : n_dt], in_=v8_h[:, :, :])
                nc.sync.dma_start(out=ident_sb, in_=id_h[:, :])

        hb_sb = consts.tile([128, n_dt, bc], f32)
        warm_sb = consts.tile([1, 1], f32)

        state = {}

        def emit_warmup(n_tr):
            # pull the activation-table load off the first-tanh critical path
            nc.scalar.activation(warm_sb, ones_f[0:1, 0:1], Tanh, bias=0.0, scale=1.0)
            # keep the PE busy until w8 lands so e_proj starts at full
            # p-state (the cost model halves PE speed for the first 3us
            # after an idle period)
            for k in range(n_tr):
                pswarm = sc_psum.tile([128, 128], f32, tag="sc")
                nc.tensor.matmul(
                    pswarm, lhsT=wd8_sb[:, 0:2, 0:128],
                    rhs=wd8_sb[:, 0:2, 0:128],
                    start=True, stop=True,
                    perf_mode=DR,
                )

        def emit_xbar_chunk(b, c):
            # transpose fp8 pairs (as u16 lanes) straight from DRAM, one
            # 512-row instruction per chunk:
            # out[p, g, q] = enc8u[b, CH*c + q, 128*g + p]
            eT = eT_pool.tile([128, n_g, CH], u16, tag="eT")
            nc.sync.dma_start(
                out=eT,
                in_=ench_h[b, CH * c : CH * (c + 1), :].bitcast(u16),
                transpose=True,
            )
            state[(b, c)] = dict(eT=eT)

        def emit_load_chunk(b, c):
            nath = nat_pool.tile([128, n_st, e_dim], fp8, tag="nath")
            nc.sync.dma_start(
                out=nath,
                in_=ench_h[b, CH * c : CH * (c + 1), :].rearrange(
                    "(j p) e -> p j e", p=128
                ),
            )
            natl = nat_pool.tile([128, n_st, e_dim], fp8, tag="natl")
            nc.sync.dma_start(
                out=natl,
                in_=encl_h[b, CH * c : CH * (c + 1), :].rearrange(
                    "(j p) e -> p j e", p=128
                ),
            )
            state[(b, c)]["nath"] = nath
            state[(b, c)]["natl"] = natl

        def emit_hproj():
            hp = sc_psum.tile([128, n_dt, bc], f32, tag="sc")
            for i in range(n_dt):
                for u in range(n_dt // 2):
                    nc.tensor.matmul(
                        hp[:, i, :],
                        lhsT=wd8_sb[:, 2 * u : 2 * u + 2, 128 * i : 128 * (i + 1)],
                        rhs=hsT8_sb[:, 2 * u : 2 * u + 2, :],
                        start=(u == 0),
                        stop=(u == n_dt // 2 - 1),
                        perf_mode=DR,
                    )
            for i in range(n_dt):
                nc.vector.tensor_scalar(
                    hb_sb[:, i, :], hp[:, i, :], 1.0 / 64,
                    ab_sb[:, i : i + 1],
                    op0=mybir.AluOpType.mult, op1=mybir.AluOpType.add,
                )

        def emit_eproj_pair(b, cp, mid_hook=None, pre_hook=None):
            eT = []
            for c in (2 * cp, 2 * cp + 1):
                # [p, g, q](u16) -> fp8 [p, g, (q two)]; per (j, g) the
                # DoubleRow rhs is [p, two, q]
                eT.append(state[(b, c)]["eT"][:, :, :].bitcast(fp8))
            th = th_pool.tile([128, n_dt, 2 * CH], fp8, tag="th")
            for i in range(n_dt):
                ps = pe_psum.tile([128, 2, CH], f32, tag="pe")
                for h in range(2):
                    for j in range(n_st):
                        for g in range(n_g):
                            rhs = eT[h][:, g, 256 * j : 256 * (j + 1)].rearrange(
                                "p (q two) -> p two q", two=2
                            )
                            nc.tensor.matmul(
                                ps[:, h, 128 * j : 128 * (j + 1)],
                                lhsT=w8_sb[:, g, :, 128 * i : 128 * (i + 1)],
                                rhs=rhs,
                                start=(g == 0),
                                stop=(g == n_g - 1),
                                perf_mode=DR,
                            )
                if mid_hook is not None:
                    # h_proj needs to land before the first tanh reads hb
                    # (program-order RAW), but after d-tile 0's matmuls so
                    # the PE ramps on e_proj while w_dec arrives
                    mid_hook()
                    mid_hook = None
                if pre_hook is not None:
                    # previous pair's scores/exp slot in here so the exp
                    # activations run before this pair's tanh queue on the
                    # Activation engine
                    pre_hook()
                    pre_hook = None
                nc.scalar.activation(
                    th[:, i, :], ps, Tanh, bias=hb_sb[:, i, b : b + 1], scale=1.0 / 64
                )
            state[(b, cp, "th")] = th

        def emit_scores(b, cp):
            th = state[(b, cp, "th")]
            scores = state[b]["scores"]
            for h in range(2):
                sc = sc_psum.tile([2, CH], f32, tag="sc")
                for m in range(n_dt // 2):
                    nc.tensor.matmul(
                        sc,
                        lhsT=v8_sb[:, :, 2 * m : 2 * m + 2],
                        rhs=th[:, 2 * m : 2 * m + 2, CH * h : CH * (h + 1)],
                        start=(m == 0),
                        stop=(m == n_dt // 2 - 1),
                        perf_mode=DR,
                    )
                pos = CH * (2 * cp + h)
                nc.scalar.activation(
                    scores[:, pos : pos + CH], sc[0:1, :], Exp, bias=0.0, scale=1.0 / 64
                )

        def emit_row_prep(b):
            maskT = sm_pool.tile([128, n_ct], bf16, tag="maskT")
            nc.sync.dma_start(out=maskT, in_=mskT_h[b, :, :])
            scores = sm_pool.tile([1, s], bf16, tag="scores")
            state[b] = dict(maskT=maskT, scores=scores)

        def emit_weighted_part(b, cp):
            # transpose this pair's exp(scores) into columns, apply the mask
            # during psum evacuation, then accumulate the (unnormalized)
            # weighted sum for the pair's two chunks. The 1/sum normalizer is
            # folded into the final output evacuation, so none of this waits
            # on a full-row softmax.
            scores = state[b]["scores"]
            maskT = state[b]["maskT"]
            half = n_ct // 2
            j0 = cp * half
            psum_at = sc_psum.tile([128, half], f32, tag="sc")
            for j in range(half):
                nc.tensor.matmul(
                    psum_at[:, j : j + 1],
                    lhsT=scores[:, 128 * (j0 + j) : 128 * (j0 + j + 1)],
                    rhs=ones_bf,
                    start=True,
                    stop=True,
                )
            if cp == 0:
                attnT_new = small_pool.tile([128, n_ct], bf16, tag="attnT")
                ah_new = small_pool.tile([128, 2, 512], fp8, tag="ah")
                al_new = small_pool.tile([128, 2, 512], fp8, tag="al")
                pw_new = w_psum.tile([1, 2, 512], f32, tag="w")
                state[b]["attnT"] = attnT_new
                state[b]["ah"] = ah_new
                state[b]["al"] = al_new
                state[b]["pw"] = pw_new
            attnT = state[b]["attnT"]
            ah, al = state[b]["ah"], state[b]["al"]
            pw = state[b]["pw"]
            nc.vector.tensor_mul(
                attnT[:, j0 : j0 + half], psum_at, maskT[:, j0 : j0 + half]
            )
            # split attn into fp8 + fp8 residual, packed by s-tile-pair parity
            # with a 512 k-stride (dual-fp8 ldweights wants wide strides);
            # ah[p, i, u] = attn(tok=(2u+i)*128+p)
            u0 = half // 2 * cp
            nu = half // 2
            asrc = attnT[:, j0 : j0 + half].rearrange("p (u two) -> p two u", two=2)
            nc.vector.tensor_copy(out=ah[:, :, u0 : u0 + nu], in_=asrc)
            nc.vector.tensor_tensor(
                out=al[:, :, u0 : u0 + nu], in0=asrc, in1=ah[:, :, u0 : u0 + nu],
                op=mybir.AluOpType.subtract,
            )
            if cp == n_chunks // 2 - 1:
                # row sum + reciprocal ahead of the weighted matmuls: the
                # tiny sum matmul would otherwise queue behind them on the
                # PE, delaying the final evacuation by the whole pair
                partials = small_pool.tile([128, 1], f32, tag="part")
                nc.vector.reduce_sum(
                    out=partials, in_=attnT, axis=mybir.AxisListType.X
                )
                psum_s = sc_psum.tile([1, 1], f32, tag="sc")
                nc.tensor.matmul(
                    psum_s, lhsT=partials, rhs=ones_f, start=True, stop=True
                )
                rsum = small_pool.tile([1, 1], f32, tag="rsum")
                nc.vector.reciprocal(rsum, psum_s)
                state[b]["rsum"] = rsum
            # hi*hi + hi*lo + lo*hi accumulate into one psum group
            # (residuals are unscaled fp8, so no rescale is needed)
            first_u, last_u = n_ct // 4 * cp, n_ct // 4 * (cp + 1) - 1
            for ec in range(n_ec):
                for u in range(first_u, last_u + 1):
                    c, jj = divmod(2 * u, n_st)
                    nath = state[(b, c)]["nath"][:, jj : jj + 2, 512 * ec : 512 * (ec + 1)]
                    natl = state[(b, c)]["natl"][:, jj : jj + 2, 512 * ec : 512 * (ec + 1)]
                    for src_a, src_e, is_first, is_last in (
                        (ah, nath, cp == 0 and u == first_u, False),
                        (ah, natl, False, False),
                        (al, nath, False, cp == n_chunks // 2 - 1 and u == last_u),
                    ):
                        nc.tensor.matmul(
                            pw[:, ec, :],
                            lhsT=src_a[:, :, u : u + 1],
                            rhs=src_e,
                            start=is_first,
                            stop=is_last,
                            perf_mode=DR,
                        )

        def emit_weighted_finish(b):
            pw = state[b]["pw"]
            rsum = state[b]["rsum"]
            out_sb = outsb_pool.tile([1, e_dim], f32, tag="outsb")
            for ec in range(n_ec):
                nc.vector.tensor_scalar_mul(
                    out_sb[:, 512 * ec : 512 * (ec + 1)],
                    pw[:, ec, :],
                    rsum[0:1, 0:1],
                )
            nc.sync.dma_start(out=out_h[b : b + 1, :], in_=out_sb)

        # ---------------- schedule ----------------
        emit_xbar_chunk(0, 0)
        emit_xbar_chunk(0, 1)
        emit_consts(0)
        emit_consts(1)
        emit_hproj()
        emit_warmup(40)
        emit_xbar_chunk(0, 2)
        emit_xbar_chunk(0, 3)
        emit_row_prep(0)
        for c in range(n_chunks):
            emit_load_chunk(0, c)
        # one-pair stagger across the whole pipeline: while the PE runs
        # eproj of pair P, it then retires scores/exp/transpose/weighted of
        # pair P-1, whose Activation-side work completed during eproj(P) --
        # the PE never waits on ScalarE.
        pairs = [(b, cp) for b in range(bc) for cp in range(n_chunks // 2)]
        for idx, (b, cp) in enumerate(pairs):
            if idx == 0:
                emit_eproj_pair(b, cp)
            else:
                pb, pcp = pairs[idx - 1]
                emit_eproj_pair(
                    b, cp,
                    pre_hook=lambda pb=pb, pcp=pcp: emit_scores(pb, pcp),
                )
                emit_weighted_part(pb, pcp)
                if pcp == n_chunks // 2 - 1:
                    emit_weighted_finish(pb)
            # loads come AFTER the retirement above (its weighted matmuls
            # free the nat ring slots these loads reuse), batched two rows
            # at a time so the copy<->transpose queue-mode switch drains
            # happen half as often
            if cp == 0:
                next_rows = [r for r in (
                    (b + 1, b + 2) if b % 2 == 0 else ()
                ) if r < bc]
                for r in next_rows:
                    for c in range(n_chunks):
                        emit_xbar_chunk(r, c)
                for r in next_rows:
                    emit_row_prep(r)
                    for c in range(n_chunks):
                        emit_load_chunk(r, c)
        pb, pcp = pairs[-1]
        emit_scores(pb, pcp)
        emit_weighted_part(pb, pcp)
        emit_weighted_finish(pb)

    nc.compile()
    return nc


_CACHE = {}


def _prep_weights(a_w, a_b, v_w, e_dim=ENC, d_dim=DEC):
    import ml_dtypes

    fp8 = ml_dtypes.float8_e4m3
    n_g, n_dt = e_dim // 256, d_dim // 128
    # w8[p, g, i, d] = 64 * a_w[DEC + 2*(128*g + p) + i, d]
    w8 = (
        (np.asarray(a_w[d_dim:], np.float32) * 64.0)
        .reshape(n_g, 128, 2, d_dim).transpose(1, 0, 2, 3).astype(fp8)
    )
    wd8 = (
        (np.asarray(a_w[:d_dim], np.float32) * 64.0)
        .reshape(n_dt, 128, d_dim).transpose(1, 0, 2).astype(fp8)
    )
    # v8[p, i, 2m+r] = 64 * v_w[(2m+i)*128 + p]  (duplicated along r: the
    # dual-fp8 ldweights wants M=2 columns)
    v8 = np.repeat(
        (np.asarray(v_w, np.float32) * 64.0)
        .reshape(n_dt // 2, 2, 128).transpose(2, 1, 0).astype(fp8)[:, :, :, None],
        2, axis=3,
    ).reshape(128, 2, n_dt)
    ab_t = np.ascontiguousarray(
        np.asarray(a_b, np.float32).reshape(n_dt, 128).T
    )
    return (
        np.ascontiguousarray(w8),
        np.ascontiguousarray(wd8),
        np.ascontiguousarray(v8),
        ab_t,
    )


def kernel(hidden_states, encoder_outputs, encoder_masks, a_w, a_b, v_w):
    import ml_dtypes
    from concourse.bass_utils import run_bass_kernel_spmd

    if "nc" not in _CACHE:
        _CACHE["nc"] = build_bass_kernel()
    nc = _CACHE["nc"]

    bf16 = ml_dtypes.bfloat16
    fp8 = ml_dtypes.float8_e4m3
    hidden_states = np.asarray(hidden_states, dtype=np.float32)
    enc_f32 = np.asarray(encoder_outputs, dtype=np.float32)
    ench = enc_f32.astype(fp8)
    encl = (enc_f32 - ench.astype(np.float32)).astype(fp8)
    encoder_masks = np.asarray(encoder_masks, dtype=np.int32)
    w8, wd8, v8, ab_t = _prep_weights(a_w, a_b, v_w)
    ident = np.eye(128, dtype=bf16)
    n_dt = DEC // 128

    in_maps = []
    for c in range(N_CORES):
        sl = slice(c * BC, (c + 1) * BC)
        hsT8 = np.ascontiguousarray(
            hidden_states[sl].T.reshape(n_dt, 128, BC).transpose(1, 0, 2)
        ).astype(fp8)
        maskT = np.ascontiguousarray(
            (encoder_masks[sl] != 0)
            .reshape(BC, S // 128, 128).transpose(0, 2, 1)
        ).astype(bf16)
        m = {
            "ench": np.ascontiguousarray(ench[sl]),
            "encl": np.ascontiguousarray(encl[sl]),
            "maskT": maskT,
            "w8": w8,
            "wd8": wd8,
            "hsT8": np.ascontiguousarray(hsT8),
            "ab_t": ab_t,
            "v8": v8,
            "ident": ident,
        }
        in_maps.append(m)

    global _LAST_IN_MAPS
    _LAST_IN_MAPS = in_maps
    res = run_bass_kernel_spmd(nc, in_maps, core_ids=list(range(N_CORES)))
    out = np.concatenate([r["out"] for r in res.results], axis=0)
    return out.astype(np.float32)


_LAST_IN_MAPS = None


# revision 38
# speedup vs baseline: 2.7736x; 1.0008x over previous
"""Bahdanau-style attention kernel for Trainium2 (8 NeuronCores, SPMD).

Math (per batch row b):
    h_proj = hidden @ a_w[:DEC]                       (DEC,)
    e_proj[s, :] = enc[s, :] @ a_w[DEC:]              (S, DEC)
    energy = tanh(e_proj + h_proj + a_b)              (S, DEC)
    scores = energy @ v_w                             (S,)
    scores = where(mask == 0, -1e10, scores)
    attn = softmax(scores)                            (S,)
    out = attn @ enc                                  (ENC,)

Sharding: data-parallel over batch (32 rows -> 4 rows on each of 8 cores);
weights replicated (pre-quantized to fp8*64 on host).

Per-core strategy:
  - The weighted sum runs in bf16 from natural-layout [tok, e] chunks
    (host-cast enc); softmax-averaging keeps per-element quantization
    error in the output, so fp8 enc there would blow the 2e-2 gate.
  - e_proj runs in fp8 with MatmulPerfMode.DoubleRow (2 k-tiles per
    instruction at 0.5 cycles/row). The transposed fp8 operand comes from
    the xbar DMA transpose moving fp8 PAIRS as uint16 lanes straight from
    DRAM: out[p, g, q](u16) = enc-pair(e=2(128g+p)(+0/1), tok q). The pair
    interleave is absorbed by the DoubleRow k-pair dimension with a
    host-permuted weight layout w8[p, g, i, d] = 64*a_w[DEC+2(128g+p)+i, d],
    so no on-chip bf16->fp8 cast and no bf16 staging is needed.
  - e_proj PSUM is [128d, 2x512tok] (a chunk pair, 2 banks) so one tanh
    activation covers 1024 tokens per d-tile, amortizing the ~185ns
    ScalarE access overhead; bias (h_proj + a_b) is per-partition.
  - scores = v . tanh as fp8 DoubleRow over d-tile pairs (v padded to
    M=2 / k-stride 512 for the dual-fp8 ldweights ISA restriction); exp
    is fused into the PSUM evacuation (scale=1/64 undoes the *64 weight
    scaling). Softmax tail on DVE.
  - The weighted sum accumulates chunk-major into one PSUM bank
    (e-halves at partitions 0/32) so nat buffers free chunk-by-chunk,
    and each iteration emits weighted(b-1) before eproj(b) so next-row
    DMA overlaps this row's PE work.
"""

import numpy as np
from contextlib import ExitStack

B, S, ENC, DEC = 32, 2048, 1024, 1024
N_CORES = 8
BC = B // N_CORES  # batch rows per core
CH = 512           # tokens per chunk


def build_bass_kernel(bc=BC, s=S, e_dim=ENC, d_dim=DEC, debug=False):
    import concourse.bass as bass
    import concourse.tile as tile
    from concourse import bacc, mybir

    f32 = mybir.dt.float32
    bf16 = mybir.dt.bfloat16
    fp8 = mybir.dt.float8e4
    u16 = mybir.dt.uint16
    i32 = mybir.dt.int32
    Tanh = mybir.ActivationFunctionType.Tanh
    Exp = mybir.ActivationFunctionType.Exp
    DR = mybir.MatmulPerfMode.DoubleRow

    assert s % (2 * CH) == 0 and e_dim % 256 == 0 and d_dim % 256 == 0
    n_chunks = s // CH             # 512-token chunks per batch row
    n_st = CH // 128               # s-tiles per chunk
    n_g = e_dim // 256             # e pair-groups (256 e-rows per group)
    n_dt = d_dim // 128            # d (output) tiles for e_proj
    n_ec = e_dim // 512            # 512-wide e chunks for the weighted sum
    n_ct = s // 128                # s-tiles per row

    nc = bacc.Bacc("TRN2", target_bir_lowering=False, debug=debug)

    ench_h = nc.dram_tensor("ench", [bc, s, e_dim], fp8, kind="ExternalInput")
    encl_h = nc.dram_tensor("encl", [bc, s, e_dim], fp8, kind="ExternalInput")
    mskT_h = nc.dram_tensor("maskT", [bc, 128, s // 128], bf16, kind="ExternalInput")
    w8_h = nc.dram_tensor("w8", [128, n_g, 2, d_dim], fp8, kind="ExternalInput")
    wd8_h = nc.dram_tensor("wd8", [128, n_dt, d_dim], fp8, kind="ExternalInput")
    hsT8_h = nc.dram_tensor("hsT8", [128, n_dt, bc], fp8, kind="ExternalInput")
    ab_h = nc.dram_tensor("ab_t", [128, n_dt], f32, kind="ExternalInput")
    v8_h = nc.dram_tensor("v8", [128, 2, n_dt], fp8, kind="ExternalInput")
    id_h = nc.dram_tensor("ident", [128, 128], bf16, kind="ExternalInput")
    out_h = nc.dram_tensor("out", [bc, e_dim], f32, kind="ExternalOutput")

    with tile.TileContext(nc) as tc, ExitStack() as ctx:
        consts = ctx.enter_context(tc.tile_pool(name="consts", bufs=1))
        nat_pool = ctx.enter_context(tc.tile_pool(name="nat", bufs=3 * n_chunks))
        eT_pool = ctx.enter_context(tc.tile_pool(name="eT", bufs=3 * n_chunks + 1))
        th_pool = ctx.enter_context(tc.tile_pool(name="th", bufs=2))
        sm_pool = ctx.enter_context(tc.tile_pool(name="softmax", bufs=3))
        small_pool = ctx.enter_context(tc.tile_pool(name="small", bufs=4))
        outsb_pool = ctx.enter_context(tc.tile_pool(name="outsb", bufs=1))
        pe_psum = ctx.enter_context(tc.tile_pool(name="pe_psum", bufs=2, space="PSUM"))
        sc_psum = ctx.enter_context(tc.tile_pool(name="sc_psum", bufs=2, space="PSUM"))
        w_psum = ctx.enter_context(tc.tile_pool(name="w_psum", bufs=1, space="PSUM"))

        # ---------------- consts ----------------
        ident_sb = consts.tile([128, 128], bf16)
        ones_bf = ident_sb[0:1, 0:1]
        ones_f = consts.tile([128, 1], f32)
        nc.vector.memset(ones_f, 1.0)

        w8_sb = consts.tile([128, n_g, 2, d_dim], fp8)
        # dual-fp8 ldweights needs a wide stride between the k-pair weight
        # blocks (walrus s3_lw_dual_fp8_restrictions rejects stride 2/4;
        # 512 verified on HW) -> stage v into a padded tile
        v8_sb = consts.tile([128, 2, 512], fp8)
        ab_sb = consts.tile([128, n_dt], f32)
        hsT8_sb = consts.tile([128, n_dt, bc], fp8)
        wd8_sb = consts.tile([128, n_dt, d_dim], fp8)

        def emit_consts(step):
            # wd8 lands before w8: the h_proj -> hb chain completes while
            # the PE is still waiting for w8 + the first transposes, so the
            # first tanh is never gated on hb
            if step == 0:
                nc.sync.dma_start(out=wd8_sb, in_=wd8_h[:, :, :])
                nc.sync.dma_start(out=hsT8_sb, in_=hsT8_h[:, :, :])
                nc.sync.dma_start(out=ab_sb, in_=ab_h[:, :])
            elif step == 1:
                nc.sync.dma_start(out=w8_sb[:, 0 : n_g // 2], in_=w8_h[:, 0 : n_g // 2])
                nc.sync.dma_start(out=w8_sb[:, n_g // 2 :], in_=w8_h[:, n_g // 2 :])
                nc.sync.dma_start(out=v8_sb[:, :, 0 : n_dt], in_=v8_h[:, :, :])
                nc.sync.dma_start(out=ident_sb, in_=id_h[:, :])

        hb_sb = consts.tile([128, n_dt, bc], f32)
        warm_sb = consts.tile([1, 1], f32)

        state = {}

        def emit_warmup(n_tr):
            # pull the activation-table load off the first-tanh critical path
            nc.scalar.activation(warm_sb, ones_f[0:1, 0:1], Tanh, bias=0.0, scale=1.0)
            # keep the PE busy until w8 lands so e_proj starts at full
            # p-state (the cost model halves PE speed for the first 3us
            # after an idle period)
            for k in range(n_tr):
                pswarm = sc_psum.tile([128, 128], f32, tag="sc")
                nc.tensor.matmul(
                    pswarm, lhsT=wd8_sb[:, 0:2, 0:128],
                    rhs=wd8_sb[:, 0:2, 0:128],
                    start=True, stop=True,
                    perf_mode=DR,
                )

        def emit_xbar_chunk(b, c):
            # transpose fp8 pairs (as u16 lanes) straight from DRAM, one
            # 512-row instruction per chunk:
            # out[p, g, q] = enc8u[b, CH*c + q, 128*g + p]
            eT = eT_pool.tile([128, n_g, CH], u16, tag="eT")
            nc.sync.dma_start(
                out=eT,
                in_=ench_h[b, CH * c : CH * (c + 1), :].bitcast(u16),
                transpose=True,
            )
            state[(b, c)] = dict(eT=eT)

        def emit_load_chunk(b, c):
            nath = nat_pool.tile([128, n_st, e_dim], fp8, tag="nath")
            nc.sync.dma_start(
                out=nath,
                in_=ench_h[b, CH * c : CH * (c + 1), :].rearrange(
                    "(j p) e -> p j e", p=128
                ),
            )
            natl = nat_pool.tile([128, n_st, e_dim], fp8, tag="natl")
            nc.sync.dma_start(
                out=natl,
                in_=encl_h[b, CH * c : CH * (c + 1), :].rearrange(
                    "(j p) e -> p j e", p=128
                ),
            )
            state[(b, c)]["nath"] = nath
            state[(b, c)]["natl"] = natl

        def emit_hproj():
            hp = sc_psum.tile([128, n_dt, bc], f32, tag="sc")
            for i in range(n_dt):
                for u in range(n_dt // 2):
                    nc.tensor.matmul(
                        hp[:, i, :],
                        lhsT=wd8_sb[:, 2 * u : 2 * u + 2, 128 * i : 128 * (i + 1)],
                        rhs=hsT8_sb[:, 2 * u : 2 * u + 2, :],
                        start=(u == 0),
                        stop=(u == n_dt // 2 - 1),
                        perf_mode=DR,
                    )
            for i in range(n_dt):
                nc.vector.tensor_scalar(
                    hb_sb[:, i, :], hp[:, i, :], 1.0 / 64,
                    ab_sb[:, i : i + 1],
                    op0=mybir.AluOpType.mult, op1=mybir.AluOpType.add,
                )

        def emit_eproj_pair(b, cp, mid_hook=None, pre_hook=None):
            eT = []
            for c in (2 * cp, 2 * cp + 1):
                # [p, g, q](u16) -> fp8 [p, g, (q two)]; per (j, g) the
                # DoubleRow rhs is [p, two, q]
                eT.append(state[(b, c)]["eT"][:, :, :].bitcast(fp8))
            th = th_pool.tile([128, n_dt, 2 * CH], fp8, tag="th")
            for i in range(n_dt):
                ps = pe_psum.tile([128, 2, CH], f32, tag="pe")
                for h in range(2):
                    for j in range(n_st):
                        for g in range(n_g):
                            rhs = eT[h][:, g, 256 * j : 256 * (j + 1)].rearrange(
                                "p (q two) -> p two q", two=2
                            )
                            nc.tensor.matmul(
                                ps[:, h, 128 * j : 128 * (j + 1)],
                                lhsT=w8_sb[:, g, :, 128 * i : 128 * (i + 1)],
                                rhs=rhs,
                                start=(g == 0),
                                stop=(g == n_g - 1),
                                perf_mode=DR,
                            )
                if mid_hook is not None:
                    # h_proj needs to land before the first tanh reads hb
                    # (program-order RAW), but after d-tile 0's matmuls so
                    # the PE ramps on e_proj while w_dec arrives
                    mid_hook()
                    mid_hook = None
                if pre_hook is not None:
                    # previous pair's scores/exp slot in here so the exp
                    # activations run before this pair's tanh queue on the
                    # Activation engine
                    pre_hook()
                    pre_hook = None
                nc.scalar.activation(
                    th[:, i, :], ps, Tanh, bias=hb_sb[:, i, b : b + 1], scale=1.0 / 64
                )
            state[(b, cp, "th")] = th

        def emit_scores(b, cp):
            th = state[(b, cp, "th")]
            scores = state[b]["scores"]
            for h in range(2):
                sc = sc_psum.tile([2, CH], f32, tag="sc")
                for m in range(n_dt // 2):
                    nc.tensor.matmul(
                        sc,
                        lhsT=v8_sb[:, :, 2 * m : 2 * m + 2],
                        rhs=th[:, 2 * m : 2 * m + 2, CH * h : CH * (h + 1)],
                        start=(m == 0),
                        stop=(m == n_dt // 2 - 1),
                        perf_mode=DR,
                    )
                pos = CH * (2 * cp + h)
                nc.scalar.activation(
                    scores[:, pos : pos + CH], sc[0:1, :], Exp, bias=0.0, scale=1.0 / 64
                )

        def emit_row_prep(b):
            maskT = sm_pool.tile([128, n_ct], bf16, tag="maskT")
            nc.sync.dma_start(out=maskT, in_=mskT_h[b, :, :])
            scores = sm_pool.tile([1, s], bf16, tag="scores")
            state[b] = dict(maskT=maskT, scores=scores)

        def emit_weighted_part(b, cp):
            # transpose this pair's exp(scores) into columns, apply the mask
            # during psum evacuation, then accumulate the (unnormalized)
            # weighted sum for the pair's two chunks. The 1/sum normalizer is
            # folded into the final output evacuation, so none of this waits
            # on a full-row softmax.
            scores = state[b]["scores"]
            maskT = state[b]["maskT"]
            half = n_ct // 2
            j0 = cp * half
            psum_at = sc_psum.tile([128, half], f32, tag="sc")
            for j in range(half):
                nc.tensor.matmul(
                    psum_at[:, j : j + 1],
                    lhsT=scores[:, 128 * (j0 + j) : 128 * (j0 + j + 1)],
                    rhs=ones_bf,
                    start=True,
                    stop=True,
                )
            if cp == 0:
                attnT_new = small_pool.tile([128, n_ct], bf16, tag="attnT")
                ah_new = small_pool.tile([128, 2, 512], fp8, tag="ah")
                al_new = small_pool.tile([128, 2, 512], fp8, tag="al")
                pw_new = w_psum.tile([1, 2, 512], f32, tag="w")
                state[b]["attnT"] = attnT_new
                state[b]["ah"] = ah_new
                state[b]["al"] = al_new
                state[b]["pw"] = pw_new
            attnT = state[b]["attnT"]
            ah, al = state[b]["ah"], state[b]["al"]
            pw = state[b]["pw"]
            nc.vector.tensor_mul(
                attnT[:, j0 : j0 + half], psum_at, maskT[:, j0 : j0 + half]
            )
            # split attn into fp8 + fp8 residual, packed by s-tile-pair parity
            # with a 512 k-stride (dual-fp8 ldweights wants wide strides);
            # ah[p, i, u] = attn(tok=(2u+i)*128+p)
            u0 = half // 2 * cp
            nu = half // 2
            asrc = attnT[:, j0 : j0 + half].rearrange("p (u two) -> p two u", two=2)
            nc.vector.tensor_copy(out=ah[:, :, u0 : u0 + nu], in_=asrc)
            nc.vector.tensor_tensor(
                out=al[:, :, u0 : u0 + nu], in0=asrc, in1=ah[:, :, u0 : u0 + nu],
                op=mybir.AluOpType.subtract,
            )
            if cp == n_chunks // 2 - 1:
                # row sum + reciprocal ahead of the weighted matmuls: the
                # tiny sum matmul would otherwise queue behind them on the
                # PE, delaying the final evacuation by the whole pair
                partials = small_pool.tile([128, 1], f32, tag="part")
                nc.vector.reduce_sum(
                    out=partials, in_=attnT, axis=mybir.AxisListType.X
                )
                psum_s = sc_psum.tile([1, 1], f32, tag="sc")
                nc.tensor.matmul(
                    psum_s, lhsT=partials, rhs=ones_f, start=True, stop=True
                )
                rsum = small_pool.tile([1, 1], f32, tag="rsum")
                nc.vector.reciprocal(rsum, psum_s)
                state[b]["rsum"] = rsum
            # hi*hi + hi*lo + lo*hi accumulate into one psum group
            # (residuals are unscaled fp8, so no rescale is needed)
            first_u, last_u = n_ct // 4 * cp, n_ct // 4 * (cp + 1) - 1
            for ec in range(n_ec):
                for u in range(first_u, last_u + 1):
                    c, jj = divmod(2 * u, n_st)
                    nath = state[(b, c)]["nath"][:, jj : jj + 2, 512 * ec : 512 * (ec + 1)]
                    natl = state[(b, c)]["natl"][:, jj : jj + 2, 512 * ec : 512 * (ec + 1)]
                    for src_a, src_e, is_first, is_last in (
                        (ah, nath, cp == 0 and u == first_u, False),
                        (ah, natl, False, False),
                        (al, nath, False, cp == n_chunks // 2 - 1 and u == last_u),
                    ):
                        nc.tensor.matmul(
                            pw[:, ec, :],
                            lhsT=src_a[:, :, u : u + 1],
                            rhs=src_e,
                            start=is_first,
                            stop=is_last,
                            perf_mode=DR,
                        )

        def emit_weighted_finish(b):
            pw = state[b]["pw"]
            rsum = state[b]["rsum"]
            out_sb = outsb_pool.tile([1, e_dim], f32, tag="outsb")
            for ec in range(n_ec):
                nc.vector.tensor_scalar_mul(
                    out_sb[:, 512 * ec : 512 * (ec + 1)],
                    pw[:, ec, :],
                    rsum[0:1, 0:1],
                )
            nc.sync.dma_start(out=out_h[b : b + 1, :], in_=out_sb)

        # ---------------- schedule ----------------
        emit_xbar_chunk(0, 0)
        emit_xbar_chunk(0, 1)
        emit_consts(0)
        emit_consts(1)
        emit_hproj()
        emit_warmup(40)
        emit_xbar_chunk(0, 2)
        emit_xbar_chunk(0, 3)
        emit_row_prep(0)
        for c in range(n_chunks):
            emit_load_chunk(0, c)
        # one-pair stagger across the whole pipeline: while the PE runs
        # eproj of pair P, it then retires scores/exp/transpose/weighted of
        # pair P-1, whose Activation-side work completed during eproj(P) --
        # the PE never waits on ScalarE.
        pairs = [(b, cp) for b in range(bc) for cp in range(n_chunks // 2)]
        for idx, (b, cp) in enumerate(pairs):
            if idx == 0:
                emit_eproj_pair(b, cp)
            else:
                pb, pcp = pairs[idx - 1]
                emit_eproj_pair(
                    b, cp,
                    pre_hook=lambda pb=pb, pcp=pcp: emit_scores(pb, pcp),
                )
                emit_weighted_part(pb, pcp)
                if pcp == n_chunks // 2 - 1:
                    emit_weighted_finish(pb)
            # loads come AFTER the retirement above (its weighted matmuls
            # free the nat ring slots these loads reuse), batched two rows
            # at a time so the copy<->transpose queue-mode switch drains
            # happen half as often
            if cp == 0:
                next_rows = [r for r in (
                    (b + 1, b + 2) if b % 2 == 0 else ()
                ) if r < bc]
                for r in next_rows:
                    for c in range(n_chunks):
                        emit_xbar_chunk(r, c)
                for r in next_rows:
                    emit_row_prep(r)
                    for c in range(n_chunks):
                        emit_load_chunk(r, c)
        pb, pcp = pairs[-1]
        emit_scores(pb, pcp)
        emit_weighted_part(pb, pcp)
        emit_weighted_finish(pb)

    nc.compile()
    return nc


_CACHE = {}


def _prep_weights(a_w, a_b, v_w, e_dim=ENC, d_dim=DEC):
    import ml_dtypes

    fp8 = ml_dtypes.float8_e4m3
    n_g, n_dt = e_dim // 256, d_dim // 128
    # w8[p, g, i, d] = 64 * a_w[DEC + 2*(128*g + p) + i, d]
    w8 = (
        (np.asarray(a_w[d_dim:], np.float32) * 64.0)
        .reshape(n_g, 128, 2, d_dim).transpose(1, 0, 2, 3).astype(fp8)
    )
    wd8 = (
        (np.asarray(a_w[:d_dim], np.float32) * 64.0)
        .reshape(n_dt, 128, d_dim).transpose(1, 0, 2).astype(fp8)
    )
    # v8[p, i, 2m+r] = 64 * v_w[(2m+i)*128 + p]  (duplicated along r: the
    # dual-fp8 ldweights wants M=2 columns)
    v8 = np.repeat(
        (np.asarray(v_w, np.float32) * 64.0)
        .reshape(n_dt // 2, 2, 128).transpose(2, 1, 0).astype(fp8)[:, :, :, None],
        2, axis=3,
    ).reshape(128, 2, n_dt)
    ab_t = np.ascontiguousarray(
        np.asarray(a_b, np.float32).reshape(n_dt, 128).T
    )
    return (
        np.ascontiguousarray(w8),
        np.ascontiguousarray(wd8),
        np.ascontiguousarray(v8),
        ab_t,
    )


def kernel(hidden_states, encoder_outputs, encoder_masks, a_w, a_b, v_w):
    import ml_dtypes
    from concourse.bass_utils import run_bass_kernel_spmd

    if "nc" not in _CACHE:
        _CACHE["nc"] = build_bass_kernel()
    nc = _CACHE["nc"]

    bf16 = ml_dtypes.bfloat16
    fp8 = ml_dtypes.float8_e4m3
    hidden_states = np.asarray(hidden_states, dtype=np.float32)
    enc_f32 = np.asarray(encoder_outputs, dtype=np.float32)
    ench = enc_f32.astype(fp8)
    encl = (enc_f32 - ench.astype(np.float32)).astype(fp8)
    encoder_masks = np.asarray(encoder_masks, dtype=np.int32)
    w8, wd8, v8, ab_t = _prep_weights(a_w, a_b, v_w)
    ident = np.eye(128, dtype=bf16)
    n_dt = DEC // 128

    in_maps = []
    for c in range(N_CORES):
        sl = slice(c * BC, (c + 1) * BC)
        hsT8 = np.ascontiguousarray(
            hidden_states[sl].T.reshape(n_dt, 128, BC).transpose(1, 0, 2)
        ).astype(fp8)
        maskT = np.ascontiguousarray(
            (encoder_masks[sl] != 0)
            .reshape(BC, S // 128, 128).transpose(0, 2, 1)
        ).astype(bf16)
        m = {
            "ench": np.ascontiguousarray(ench[sl]),
            "encl": np.ascontiguousarray(encl[sl]),
            "maskT": maskT,
            "w8": w8,
            "wd8": wd8,
            "hsT8": np.ascontiguousarray(hsT8),
            "ab_t": ab_t,
            "v8": v8,
            "ident": ident,
        }
        in_maps.append(m)

    global _LAST_IN_MAPS
    _LAST_IN_MAPS = in_maps
    res = run_bass_kernel_spmd(nc, in_maps, core_ids=list(range(N_CORES)))
    out = np.concatenate([r["out"] for r in res.results], axis=0)
    return out.astype(np.float32)


_LAST_IN_MAPS = None
